# revision 39
# baseline (speedup 1.0000x reference)
"""AttentionGAT (2-layer GAT + attention fusion gate + mean-pool + MLP head)
as a Bass/Tile kernel on 8 Trainium2 NeuronCores.

v3: prefetch-pipelined dma_gathers round-robined over the 4 SWDGE queues so
descriptor generation overlaps 4-wide across Q7 core pairs; deeper tile
pools to keep the gather dispatch off the compute-completion critical path;
AllGather chunking with a small tail chunk.

Row layout (haug, 512 fp8 bytes): [0:128]=16*h0 |128|=1.0 [129:257]=16*h1
|257|=1.0; bf16 cols 129,130 (bytes 258:262) = 64*as0, 64*as1; fp8 bytes
262,263 = 64*ad0, 64*ad1.  h2 rows (256B): [0:128]=16*h2 |128|=1.0; bf16
col 65 (bytes 130:132) = 64*as2; fp8 byte 132 = 64*ad2.
"""

import os
import sys
import types

sys.path.insert(0, "/opt/trn_rl_repo")

import numpy as np
import ml_dtypes

import concourse.bass as bass
import concourse.mybir as mybir
import concourse.tile as tile
from concourse import bacc
from concourse import library_config
from concourse.bass_utils import run_bass_kernel_spmd

BF16 = ml_dtypes.bfloat16
FP8 = ml_dtypes.float8_e4m3
NCORES = 8
LAST_EXEC_NS = None  # set when AGAT_PROFILE=1

SH1 = 2.0   # scale on h columns of W1
SH2 = 4.0   # scale on h columns of W2
SA = 64.0   # scale on as/ad projection columns
SG = 32.0   # scale on gate (attn) columns

# AllGather chunk sizes in 128-row tiles per core (small chunks fire early
# during the phase; one mid-size tail chunk).
AG_CHUNKS = [3, 3, 3, 3, 3, 3, 3, 3, 6]
# h2 AllGather chunks: ~1.6MB-out collectives sustain 70-100GB/s while one
# big 7.9MB AG degrades to 22GB/s. All but the last issue late in phase 2a
# (past the last gather dispatch, so their waits block nothing).
H2_CHUNKS = [5, 5, 5, 5, 5, 5]


def _install_ntff_hook():
    try:
        from antenv.axon_hooks import get_axon_ntff_profile_hook  # noqa: F401
        return
    except ImportError:
        pass
    try:
        import antenv
        from trn_agent_boot.trn_boot import _ntff_profile_via_ctypes

        mod = types.ModuleType("antenv.axon_hooks")
        _h = [None]
        mod.set_axon_ntff_profile_hook = lambda h: _h.__setitem__(0, h)
        mod.get_axon_ntff_profile_hook = lambda: _h[0]
        sys.modules["antenv.axon_hooks"] = mod
        antenv.axon_hooks = mod
        mod.set_axon_ntff_profile_hook(
            _ntff_profile_via_ctypes("/opt/axon/libaxon_pjrt.so")
        )
    except Exception:
        pass


def _wrap_idx(a):
    """dma_gather index layout: idx i at [i%16, i//16], tiled to 128 parts."""
    return np.tile(a.reshape(-1, 16).T, (8, 1)).astype(np.int16)


def build_program(cfg):
    IN_DIM, HOG = cfg["in_dim"], cfg["hog"]
    NPC, NBLK, NT1, NG = cfg["npc"], cfg["nblk"], cfg["nt1"], cfg["ng"]
    NPAD = NPC * NCORES
    NPAIR = NT1 // 2
    NC1 = 264          # h0 |z| h1 |z| as0 as1 ad0 ad1 | gate0 gate1
    ROW1, ROW2 = 512, 256
    MG = 3             # node-tiles per phase-1 PSUM group

    # phase-1 K tiling: DoubleRow pairs over the hog region, then singles
    KPAIRS = [(k, k + 256) for k in range(0, HOG - 255, 256)]
    ksing = []
    kp_end = KPAIRS[-1][1] if KPAIRS else 0
    if kp_end < HOG:
        ksing.append((kp_end, HOG))        # hog remainder (psA)
    ksing.append((HOG, IN_DIM))            # cov (psB)

    dt = mybir.dt
    DR = mybir.MatmulPerfMode.DoubleRow
    nc = bacc.Bacc("TRN2", target_bir_lowering=False, debug=False,
                   num_devices=NCORES, num_swdge_queues=4)

    xT = nc.dram_tensor("xT", [IN_DIM, NPC], dt.float8e4, kind="ExternalInput").ap()
    w1 = nc.dram_tensor("w1", [IN_DIM, NC1], dt.float8e4, kind="ExternalInput").ap()
    w2 = nc.dram_tensor("w2", [2, 128, 132], dt.bfloat16, kind="ExternalInput").ap()
    b1b = nc.dram_tensor("b1b", [128, 256], dt.float32, kind="ExternalInput").ap()
    b2b = nc.dram_tensor("b2b", [128, 128], dt.float32, kind="ExternalInput").ap()
    abb = nc.dram_tensor("abb", [128, 2], dt.float32, kind="ExternalInput").ap()
    identin = nc.dram_tensor("identin", [128, 128], dt.bfloat16, kind="ExternalInput").ap()
    gidx = nc.dram_tensor("gidx", [NBLK, 128, NT1 * 8], dt.int16, kind="ExternalInput").ap()
    gidx2 = nc.dram_tensor("gidx2", [NBLK, 128, NT1 * 8], dt.int16, kind="ExternalInput").ap()
    otp = nc.dram_tensor("otp", [NBLK, 128, NT1 * 128], dt.float8e4, kind="ExternalInput").ap()
    saT = nc.dram_tensor("saT", [NBLK, 128, NT1 * 128], dt.float8e4, kind="ExternalInput").ap()
    bhot = nc.dram_tensor("bhot", [NBLK, 128, NG], dt.bfloat16, kind="ExternalInput").ap()
    pout = nc.dram_tensor("pout", [NG, 128], dt.float32, kind="ExternalOutput").ap()

    AOT = mybir.AluOpType
    AFT = mybir.ActivationFunctionType

    # AG chunk boundaries (in tiles)
    ag_starts = []
    s = 0
    for czs in AG_CHUNKS:
        ag_starts.append(s)
        s += czs
    assert s == NBLK

    with tile.TileContext(nc) as tc:
        with (
            tc.tile_pool(name="constp", bufs=1) as constp,
            tc.tile_pool(name="dramp", bufs=1, space="DRAM") as dramp,
        ):
            haug_sh = dramp.tile([NPC, ROW1], dt.float8e4)
            haug = dramp.tile([NPAD, ROW1], dt.float8e4)
            h2_sh = dramp.tile([NPC, ROW2], dt.float8e4)
            h2f = dramp.tile([NPAD, ROW2], dt.float8e4)

            nc.gpsimd.load_library(library_config.mlp)
            ident = constp.tile([128, 128], dt.bfloat16)
            nc.sync.dma_start(ident[:], identin[:])
            b1_sb = constp.tile([128, 256], dt.float32)
            nc.sync.dma_start(b1_sb[:], b1b[:])
            b2_sb = constp.tile([128, 128], dt.float32)
            nc.sync.dma_start(b2_sb[:], b2b[:])
            ab_sb = constp.tile([128, 2], dt.float32)
            nc.sync.dma_start(ab_sb[:], abb[:])
            w2_sb = []
            for kk in range(2):
                t = constp.tile([128, 132], dt.bfloat16, tag=f"w2_{kk}",
                                name=f"w2sb{kk}")
                nc.sync.dma_start(t[:], w2[kk])
                w2_sb.append(t)
            # w1 tiles: pairs [128, 2, 264] + singles
            w1p_sb = []
            for k, (k0, k1) in enumerate(KPAIRS):
                t = constp.tile([128, 2, NC1], dt.float8e4, tag=f"w1p_{k}",
                                name=f"w1p{k}")
                nc.sync.dma_start(t[:, 0, :], w1[k0:k0 + 128, :])
                nc.sync.dma_start(t[:, 1, :], w1[k0 + 128:k1, :])
                w1p_sb.append(t)
            w1s_sb = []
            for k, (k0, k1) in enumerate(ksing):
                t = constp.tile([k1 - k0, NC1], dt.float8e4, tag=f"w1s_{k}",
                                name=f"w1s{k}")
                nc.sync.dma_start(t[:], w1[k0:k1, :])
                w1s_sb.append(t)

            # ---------------- phase 1: h_aug for own node shard ------------
            with (
                tc.tile_pool(name="p1x", bufs=1) as p1x,
                tc.tile_pool(name="p1o", bufs=2) as p1o,
                tc.tile_pool(name="p1ps", bufs=1, space="PSUM") as p1ps,
            ):
                NM = NPC // 128
                # resident xT shard in SBUF (one full-row load per k-slice)
                xp = []
                for k, (k0, k1) in enumerate(KPAIRS):
                    t = p1x.tile([128, 2, NPC], dt.float8e4, tag=f"xp{k}",
                                 name=f"xp{k}")
                    nc.sync.dma_start(t[:, 0, :], xT[k0:k0 + 128, :])
                    nc.sync.dma_start(t[:, 1, :], xT[k0 + 128:k1, :])
                    xp.append(t)
                xs = []
                for k, (k0, k1) in enumerate(ksing):
                    t = p1x.tile([k1 - k0, NPC], dt.float8e4, tag=f"xs{k}",
                                 name=f"xs{k}")
                    nc.sync.dma_start(t[:], xT[k0:k1, :])
                    xs.append(t)
                done_chunks = set()
                for g0m in range(0, NM, MG):
                    ms = list(range(g0m, min(g0m + MG, NM)))
                    nms = len(ms)
                    c0 = ms[0] * 128
                    psA = [p1ps.tile([128, NC1], dt.float32, tag=f"A{i}",
                                     name=f"psA{g0m}_{i}") for i in range(nms)]
                    psB = [p1ps.tile([128, NC1], dt.float32, tag=f"B{i}",
                                     name=f"psB{g0m}_{i}") for i in range(nms)]
                    for k, (k0, k1) in enumerate(KPAIRS):
                        for i in range(nms):
                            o0 = c0 + i * 128
                            nc.tensor.matmul(
                                psA[i][:],
                                xp[k][:, :, o0:o0 + 128],
                                w1p_sb[k][:],
                                start=(k == 0), stop=False,
                                perf_mode=DR,
                            )
                    for k, (k0, k1) in enumerate(ksing):
                        cov = (k == len(ksing) - 1)
                        for i in range(nms):
                            o0 = c0 + i * 128
                            nc.tensor.matmul(
                                (psB[i] if cov else psA[i])[:],
                                xs[k][:, o0:o0 + 128],
                                w1s_sb[k][:],
                                start=cov, stop=True,
                            )
                    for i, m in enumerate(ms):
                        lg = p1o.tile([128, 2], dt.float32, tag="lg", name=f"lg{m}")
                        nc.vector.tensor_copy(lg[:], psA[i][:, 262:264])
                        nc.vector.tensor_tensor(lg[:], lg[:],
                                                psB[i][:, 262:264], AOT.add)
                        nc.vector.scalar_tensor_tensor(
                            lg[:], lg[:], 1.0 / SG, ab_sb[:], AOT.mult, AOT.add)
                        mx = p1o.tile([128, 1], dt.float32, tag="mx", name=f"mx{m}")
                        nc.vector.tensor_reduce(mx[:], lg[:],
                                                mybir.AxisListType.X, AOT.max)
                        mxn = p1o.tile([128, 1], dt.float32, tag="mxn", name=f"mxn{m}")
                        nc.vector.tensor_scalar(mxn[:], mx[:], -1.0, None, AOT.mult)
                        em = p1o.tile([128, 2], dt.float32, tag="em", name=f"em{m}")
                        nc.scalar.activation(em[:], lg[:], AFT.Exp, bias=mxn[:, 0:1])
                        sm = p1o.tile([128, 1], dt.float32, tag="sm", name=f"sm{m}")
                        nc.vector.tensor_reduce(sm[:], em[:],
                                                mybir.AxisListType.X, AOT.add)
                        rs = p1o.tile([128, 1], dt.float32, tag="rs", name=f"rs{m}")
                        nc.vector.reciprocal(rs[:], sm[:])
                        gg = p1o.tile([128, 2], dt.float32, tag="gg", name=f"gg{m}")
                        nc.vector.tensor_scalar(gg[:], em[:], rs[:, 0:1], None,
                                                AOT.mult)
                        h1 = p1o.tile([128, 262], dt.float32, tag="h1", name=f"h1{m}")
                        tmb = p1o.tile([128, 262], dt.float32, tag="tmb", name=f"tmb{m}")
                        nc.scalar.mul(h1[:], psA[i][:, 0:262], gg[:, 0:1])
                        nc.scalar.mul(tmb[:], psB[i][:, 0:262], gg[:, 1:2])
                        nc.vector.tensor_tensor(h1[:], h1[:], tmb[:], AOT.add)
                        ha = p1o.tile([128, ROW1], dt.float8e4, tag="ha", name=f"ha{m}")
                        nc.vector.tensor_copy(ha[:, 0:258], h1[:, 0:258])
                        nc.vector.memset(ha[:, 128:129], 1.0)
                        nc.vector.memset(ha[:, 257:258], 1.0)
                        hab = ha[:].bitcast(dt.bfloat16)
                        nc.vector.tensor_copy(hab[:, 129:131], h1[:, 258:260])
                        nc.vector.tensor_copy(ha[:, 262:264], h1[:, 260:262])
                        nc.sync.dma_start(haug_sh[m * 128:(m + 1) * 128, :], ha[:])
                    done_m = ms[-1] + 1
                    for ci, (cs, cz) in enumerate(zip(ag_starts, AG_CHUNKS)):
                        if ci in done_chunks or cs + cz > done_m:
                            continue
                        done_chunks.add(ci)
                        nc.gpsimd.collective_compute(
                            "AllGather", AOT.bypass,
                            replica_groups=[list(range(NCORES))],
                            ins=[haug_sh[cs * 128:(cs + cz) * 128, :]],
                            outs=[haug[cs * 128 * NCORES:
                                       (cs + cz) * 128 * NCORES, :]],
                        )

            # ---------------- phase 2a: layer-1 aggregation -> h2_aug ------
            PF = 4   # gather prefetch depth
            LF = 2   # input-tile (sat/ot/adt) prefetch depth
            with (
                tc.tile_pool(name="p2G", bufs=PF + 3) as p2G,
                tc.tile_pool(name="p2i", bufs=PF + 2) as p2i,
                tc.tile_pool(name="p2in", bufs=LF + 2) as p2in,
                tc.tile_pool(name="p2s", bufs=2) as p2s,
                tc.tile_pool(name="p2o", bufs=2) as p2o,
                tc.tile_pool(name="p2ps", bufs=1, space="PSUM") as p2ps,
            ):
                tiles = {}
                ins2 = {}

                def issue_2a(j):
                    osb = p2i.tile([128, NT1 * 8], dt.int16, tag="osb",
                                   name=f"osb{j}")
                    nc.sync.dma_start(osb[:], gidx[j])
                    G = p2G.tile([128, NT1, ROW1], dt.float8e4, tag="G",
                                 name=f"G{j}")
                    nc.gpsimd.dma_gather(
                        G[:, :, :], haug[:, :], osb[:],
                        NT1 * 128, NT1 * 128, ROW1, single_packet=False,
                        queue_num=j % 4)
                    tiles[j] = (osb, G)

                def load_2a(j):
                    sat = p2in.tile([128, NT1 * 128], dt.float8e4, tag="sat",
                                    name=f"sat{j}")
                    nc.sync.dma_start(sat[:], saT[j])
                    ot = p2in.tile([128, NT1 * 128], dt.float8e4, tag="ot",
                                   name=f"ot{j}")
                    nc.sync.dma_start(ot[:], otp[j])
                    adt = p2in.tile([128, 2], dt.float8e4, tag="adt",
                                    name=f"adt{j}")
                    nc.sync.dma_start(adt[:],
                                      haug_sh[j * 128:(j + 1) * 128, 262:264])
                    ins2[j] = (sat, ot, adt)

                for j in range(PF):
                    issue_2a(j)
                for j in range(LF):
                    load_2a(j)
                for j in range(NBLK):
                    if j + PF < NBLK:
                        issue_2a(j + PF)
                    if j + LF < NBLK:
                        load_2a(j + LF)
                    if j == NBLK - 2:
                        # h2 AG chunks 0..n-2: all later gather issues are
                        # done, so their waits block nothing on the queue
                        cs = 0
                        for cz in H2_CHUNKS[:-1]:
                            nc.gpsimd.collective_compute(
                                "AllGather", AOT.bypass,
                                replica_groups=[list(range(NCORES))],
                                ins=[h2_sh[cs * 128:(cs + cz) * 128, :]],
                                outs=[h2f[cs * 128 * NCORES:
                                          (cs + cz) * 128 * NCORES, :]],
                            )
                            cs += cz
                    osb, G = tiles.pop(j)
                    sat, ot, adt = ins2.pop(j)
                    Gb = G[:].bitcast(dt.bfloat16)
                    adps = p2ps.tile([128, 2 * NT1], dt.float32, tag="adps",
                                     name=f"adps{j}")
                    for t in range(NT1):
                        nc.tensor.matmul(adps[:, 2 * t:2 * t + 2],
                                         ot[:, t * 128:(t + 1) * 128],
                                         adt[:, :], start=True, stop=True)
                    ade = p2s.tile([128, 2 * NT1], dt.float32, tag="ade",
                                   name=f"ade{j}")
                    nc.scalar.mul(ade[:], adps[:], 1.0)
                    es = []
                    for h in (0, 1):
                        z = p2s.tile([128, NT1], dt.float32, tag=f"z{h}",
                                     name=f"z{h}_{j}")
                        nc.vector.tensor_tensor(z[:], Gb[:, :, 129 + h],
                                                ade[:, h::2], AOT.add)
                        nc.vector.scalar_tensor_tensor(
                            z[:], z[:], 0.2, z[:], AOT.mult, AOT.max)
                        e = p2s.tile([128, NT1], dt.float32, tag=f"e{h}",
                                     name=f"e{h}_{j}")
                        nc.scalar.activation(e[:], z[:], AFT.Exp, scale=1.0 / SA)
                        es.append(e)
                    # fold es into G head columns in-place (incl. the 1.0
                    # marker column -> denominator comes out of the matmul)
                    for h in (0, 1):
                        gsl = G[:, :, 129 * h:129 * h + 129]
                        nc.vector.tensor_tensor(
                            gsl, gsl,
                            es[h][:].unsqueeze(2).broadcast_to([128, NT1, 129]),
                            AOT.mult)
                    accF = [p2ps.tile([128, 129], dt.float32, tag=f"F{h}",
                                      name=f"F{h}_{j}", bufs=2) for h in (0, 1)]
                    for pi in range(NPAIR):
                        t = 2 * pi
                        sa2 = sat[:, t * 128:(t + 2) * 128].rearrange(
                            "p (k d) -> p k d", k=2)
                        for h in (0, 1):
                            nc.tensor.matmul(
                                accF[h][:], sa2,
                                G[:, t:t + 2, 129 * h:129 * h + 129],
                                start=(pi == 0), stop=(pi == NPAIR - 1),
                                perf_mode=DR)
                    hr = p2o.tile([128, 256], dt.float32, tag="hr", name=f"hr{j}")
                    for h in (0, 1):
                        den = p2o.tile([128, 1], dt.float32, tag=f"den{h}",
                                       name=f"den{h}_{j}")
                        nc.vector.tensor_scalar(den[:], accF[h][:, 128:129],
                                                SH1, SH1 * 1e-6, AOT.mult, AOT.add)
                        rcp = p2o.tile([128, 1], dt.float32, tag=f"rcp{h}",
                                       name=f"rcp{h}_{j}")
                        nc.vector.reciprocal(rcp[:], den[:])
                        nc.scalar.mul(hr[:, h * 128:(h + 1) * 128],
                                      accF[h][:, 0:128], rcp[:, 0:1])
                    nc.vector.tensor_tensor(hr[:], hr[:], b1_sb[:], AOT.add)
                    hrb = p2o.tile([128, 256], dt.bfloat16, tag="hrb", name=f"hrb{j}")
                    nc.scalar.activation(hrb[:], hr[:], AFT.Relu)
                    h2ps = p2ps.tile([128, 132], dt.float32, tag="h2ps",
                                     name=f"h2ps{j}")
                    for kk in range(2):
                        trp = p2ps.tile([128, 128], dt.bfloat16, tag="trp",
                                        name=f"trp{j}_{kk}")
                        nc.tensor.transpose(trp[:],
                                            hrb[:, kk * 128:(kk + 1) * 128],
                                            ident[:])
                        trs = p2s.tile([128, 128], dt.bfloat16, tag="trs",
                                       name=f"trs{j}_{kk}")
                        nc.scalar.mul(trs[:], trp[:], 1.0)
                        nc.tensor.matmul(h2ps[:], trs[:], w2_sb[kk][:],
                                         start=(kk == 0), stop=(kk == 1))
                    h2a = p2o.tile([128, ROW2], dt.float8e4, tag="h2a",
                                   name=f"h2a{j}")
                    nc.scalar.mul(h2a[:, 0:128], h2ps[:, 0:128], 1.0)
                    nc.vector.memset(h2a[:, 128:129], 1.0)
                    h2ab = h2a[:].bitcast(dt.bfloat16)
                    nc.vector.tensor_copy(h2ab[:, 65:66], h2ps[:, 129:130])
                    nc.vector.tensor_copy(h2a[:, 132:133], h2ps[:, 130:131])
                    nc.sync.dma_start(h2_sh[j * 128:(j + 1) * 128, :], h2a[:])
                # h2 AG tail chunk
                cs = sum(H2_CHUNKS[:-1])
                cz = H2_CHUNKS[-1]
                nc.gpsimd.collective_compute(
                    "AllGather", AOT.bypass,
                    replica_groups=[list(range(NCORES))],
                    ins=[h2_sh[cs * 128:(cs + cz) * 128, :]],
                    outs=[h2f[cs * 128 * NCORES:
                              (cs + cz) * 128 * NCORES, :]],
                )

            # ---------------- phase 2b: layer-2 aggregation + pooling ------
            PF2 = 4
            LF2 = 2
            with (
                tc.tile_pool(name="p3G", bufs=PF2 + 3) as p3G,
                tc.tile_pool(name="p3i", bufs=PF2 + 2) as p3i,
                tc.tile_pool(name="p3in", bufs=LF2 + 2) as p3in,
                tc.tile_pool(name="p3a", bufs=1) as p3a,
                tc.tile_pool(name="p3ot", bufs=4) as p3ot,
                tc.tile_pool(name="p3s", bufs=2) as p3s,
                tc.tile_pool(name="p3o", bufs=2) as p3o,
                tc.tile_pool(name="p3ps", bufs=1, space="PSUM") as p3ps,
                tc.tile_pool(name="poolps", bufs=1, space="PSUM") as poolps,
            ):
                tiles3 = {}
                ins3 = {}
                ades = {}

                def issue_2b(j):
                    osb = p3i.tile([128, NT1 * 8], dt.int16, tag="osb",
                                   name=f"osb3_{j}")
                    nc.sync.dma_start(osb[:], gidx2[j])
                    G = p3G.tile([128, NT1, ROW2], dt.float8e4, tag="G2",
                                 name=f"G2_{j}")
                    nc.gpsimd.dma_gather(
                        G[:, :, :], h2f[:, :], osb[:],
                        NT1 * 128, NT1 * 128, ROW2, single_packet=False,
                        queue_num=j % 4)
                    tiles3[j] = (osb, G)

                def load_2b(j):
                    sat = p3in.tile([128, NT1 * 128], dt.float8e4, tag="sat",
                                    name=f"sat3_{j}")
                    nc.sync.dma_start(sat[:], saT[j])
                    bh = p3in.tile([128, NG], dt.bfloat16, tag="bh", name=f"bh{j}")
                    nc.sync.dma_start(bh[:], bhot[j])
                    ins3[j] = (sat, bh)

                # ade precompute for ALL blocks: depends only on otp + h2_sh
                # (local), so it runs on Tensor/Scalar/Sync underneath the h2
                # AllGather chain, and moves the 0.6MB/block ot loads out of
                # the gather steady state.
                pool_ps = poolps.tile([NG, 128], dt.float32)
                for j in range(NBLK):
                    ot = p3ot.tile([128, NT1 * 128], dt.float8e4, tag="ot",
                                   name=f"ot3_{j}")
                    nc.sync.dma_start(ot[:], otp[j])
                    adt = p3ot.tile([128, 1], dt.float8e4, tag="adt",
                                    name=f"adt3_{j}")
                    nc.sync.dma_start(adt[:],
                                      h2_sh[j * 128:(j + 1) * 128, 132:133])
                    adps = p3ps.tile([128, NT1], dt.float32, tag="adps",
                                     name=f"adps3_{j}", bufs=2)
                    for t in range(NT1):
                        nc.tensor.matmul(adps[:, t:t + 1],
                                         ot[:, t * 128:(t + 1) * 128],
                                         adt[:, :], start=True, stop=True)
                    ade = p3a.tile([128, NT1], dt.float32, tag=f"ade{j}",
                                   name=f"ade3_{j}")
                    nc.scalar.mul(ade[:], adps[:], 1.0)
                    ades[j] = ade
                for j in range(PF2):
                    issue_2b(j)
                for j in range(LF2):
                    load_2b(j)
                for j in range(NBLK):
                    if j + PF2 < NBLK:
                        issue_2b(j + PF2)
                    if j + LF2 < NBLK:
                        load_2b(j + LF2)
                    osb, G = tiles3.pop(j)
                    sat, bh = ins3.pop(j)
                    ade = ades.pop(j)
                    Gb = G[:].bitcast(dt.bfloat16)
                    z = p3s.tile([128, NT1], dt.float32, tag="z", name=f"z3_{j}")
                    nc.vector.tensor_tensor(z[:], Gb[:, :, 65], ade[:], AOT.add)
                    nc.vector.scalar_tensor_tensor(
                        z[:], z[:], 0.2, z[:], AOT.mult, AOT.max)
                    e = p3s.tile([128, NT1], dt.float32, tag="e", name=f"e3_{j}")
                    nc.scalar.activation(e[:], z[:], AFT.Exp, scale=1.0 / SA)
                    gsl = G[:, :, 0:129]
                    nc.vector.tensor_tensor(
                        gsl, gsl,
                        e[:].unsqueeze(2).broadcast_to([128, NT1, 129]),
                        AOT.mult)
                    accF = p3ps.tile([128, 129], dt.float32, tag="F",
                                     name=f"F3_{j}", bufs=2)
                    for pi in range(NPAIR):
                        t = 2 * pi
                        sa2 = sat[:, t * 128:(t + 2) * 128].rearrange(
                            "p (k d) -> p k d", k=2)
                        nc.tensor.matmul(accF[:], sa2, G[:, t:t + 2, 0:129],
                                         start=(pi == 0), stop=(pi == NPAIR - 1),
                                         perf_mode=DR)
                    den = p3o.tile([128, 1], dt.float32, tag="den", name=f"den3_{j}")
                    nc.vector.tensor_scalar(den[:], accF[:, 128:129], SH2,
                                            SH2 * 1e-6, AOT.mult, AOT.add)
                    rcp = p3o.tile([128, 1], dt.float32, tag="rcp", name=f"rcp3_{j}")
                    nc.vector.reciprocal(rcp[:], den[:])
                    ov = p3o.tile([128, 128], dt.float32, tag="ov", name=f"ov{j}")
                    nc.scalar.mul(ov[:], accF[:, 0:128], rcp[:, 0:1])
                    nc.vector.tensor_tensor(ov[:], ov[:], b2_sb[:], AOT.add)
                    ob = p3o.tile([128, 128], dt.bfloat16, tag="ob", name=f"ob{j}")
                    nc.scalar.activation(ob[:], ov[:], AFT.Relu)
                    nc.tensor.matmul(pool_ps[:], bh[:], ob[:],
                                     start=(j == 0), stop=(j == NBLK - 1))
                pc = p3o.tile([NG, 128], dt.float32, tag="pc")
                nc.vector.tensor_copy(pc[:], pool_ps[:])
                nc.sync.dma_start(pout[:], pc[:])

    nc.compile()
    return nc


def prepare_inputs(inputs, cfg):
    """Host-side sharding/layout. Returns in_maps (one dict per core)."""
    IN_DIM, HOG = cfg["in_dim"], cfg["hog"]
    N, NPC, NBLK, NT1, NG = cfg["n"], cfg["npc"], cfg["nblk"], cfg["nt1"], cfg["ng"]
    NPAD = NPC * NCORES

    x = np.asarray(inputs["x"], np.float32)
    ei = np.asarray(inputs["edge_index"])
    batch = np.asarray(inputs["batch"]).astype(np.int64)
    W1 = np.asarray(inputs["W1"], np.float32)
    a_src1 = np.asarray(inputs["a_src1"], np.float32)
    a_dst1 = np.asarray(inputs["a_dst1"], np.float32)
    W2 = np.asarray(inputs["W2"], np.float32)
    a_src2 = np.asarray(inputs["a_src2"], np.float32)
    a_dst2 = np.asarray(inputs["a_dst2"], np.float32)
    attn_W = np.asarray(inputs["attn_W"], np.float32)
    attn_b = np.asarray(inputs["attn_b"], np.float32)
    b1 = np.asarray(inputs["b1"], np.float32)
    b2 = np.asarray(inputs["b2"], np.float32)

    # augmented weights (scale-folded)
    w1aug = np.zeros((IN_DIM, 264), np.float32)
    w1aug[:, 0:128] = W1[:, 0:128] * SH1
    w1aug[:, 129:257] = W1[:, 128:256] * SH1
    w1aug[:, 258] = W1[:, 0:128] @ a_src1[0] * SA
    w1aug[:, 259] = W1[:, 128:256] @ a_src1[1] * SA
    w1aug[:, 260] = W1[:, 0:128] @ a_dst1[0] * SA
    w1aug[:, 261] = W1[:, 128:256] @ a_dst1[1] * SA
    w1aug[:, 262:264] = attn_W * SG
    w2aug = np.zeros((256, 132), np.float32)
    w2aug[:, 0:128] = W2 * SH2
    w2aug[:, 129] = W2 @ a_src2[0] * SA
    w2aug[:, 130] = W2 @ a_dst2[0] * SA

    xT = np.zeros((IN_DIM, NPAD), FP8)
    xT[:, :N] = np.ascontiguousarray(x.T).astype(FP8)

    # edges sorted by destination, self loops appended
    idt = ei.dtype
    src = np.concatenate([ei[0], np.arange(N, dtype=idt)]).astype(np.int64)
    dst = np.concatenate([ei[1], np.arange(N, dtype=idt)]).astype(np.int64)
    order = np.argsort(dst, kind="stable")
    src_s, dst_s = src[order], dst[order]
    nblk_g = NPAD // 128
    L = NT1 * 128
    cnt = np.bincount(dst_s // 128, minlength=nblk_g)
    assert cnt.max() <= L, (cnt.max(), L)
    offs = np.concatenate([[0], np.cumsum(cnt)])
    sidx_all = np.zeros((nblk_g, L), np.int64)
    dloc_all = np.full((nblk_g, L), -1.0, np.float32)
    for b in range(nblk_g):
        s, e = offs[b], offs[b + 1]
        n = e - s
        sidx_all[b, :n] = src_s[s:e]
        dloc_all[b, :n] = (dst_s[s:e] - 128 * b).astype(np.float32)

    # full one-hot (transposed): otp[b, dl, e] = 1
    otp_all = np.zeros((nblk_g, 128, L), FP8)
    # Sa one-hot in edge-tile layout: saT[b, e%128, (e//128)*128 + dl] = 1
    saT_all = np.zeros((nblk_g, 128, L), FP8)
    eidx = np.arange(L)
    for b in range(nblk_g):
        m = dloc_all[b] >= 0
        dlv = dloc_all[b][m].astype(np.int64)
        ev = eidx[m]
        otp_all[b, dlv, ev] = 1.0
        saT_all[b, ev % 128, (ev // 128) * 128 + dlv] = 1.0

    bh_all = np.zeros((nblk_g, 128, NG), np.float32)
    for b in range(nblk_g):
        base = 128 * b
        hi = min(N - base, 128)
        if hi > 0:
            bh_all[b, np.arange(hi), batch[base:base + hi]] = 1.0

    ident_t = np.eye(128, dtype=np.float32).astype(BF16)
    b1b = np.tile(b1[None, :], (128, 1)).astype(np.float32)
    b2b = np.tile(b2[None, :], (128, 1)).astype(np.float32)
    abb = np.tile(attn_b[None, :], (128, 1)).astype(np.float32)

    # chunk-major AllGather layout with chunk sizes AG_CHUNKS (in tiles):
    # node n (core c, local row r, local tile m=r//128, chunk ci) ->
    #   row 128*(cs*8 + c*cz) + (r - cs*128)
    tile_chunk = np.zeros(NBLK, np.int64)
    chunk_start = np.zeros(len(AG_CHUNKS), np.int64)
    s = 0
    for ci, cz in enumerate(AG_CHUNKS):
        chunk_start[ci] = s
        tile_chunk[s:s + cz] = ci
        s += cz
    n_arr = np.arange(NPAD, dtype=np.int64)
    cc_, rr_ = n_arr // NPC, n_arr % NPC
    mm_ = rr_ // 128
    ci_ = tile_chunk[mm_]
    cs_ = chunk_start[ci_]
    cz_ = np.asarray(AG_CHUNKS, np.int64)[ci_]
    rowmap = 128 * (cs_ * NCORES + cc_ * cz_) + (rr_ - cs_ * 128)
    # h2f chunk-major layout per H2_CHUNKS
    tile_chunk2 = np.zeros(NBLK, np.int64)
    chunk_start2 = np.zeros(len(H2_CHUNKS), np.int64)
    s = 0
    for ci, cz in enumerate(H2_CHUNKS):
        chunk_start2[ci] = s
        tile_chunk2[s:s + cz] = ci
        s += cz
    ci2_ = tile_chunk2[mm_]
    cs2_ = chunk_start2[ci2_]
    cz2_ = np.asarray(H2_CHUNKS, np.int64)[ci2_]
    rowmap2 = 128 * (cs2_ * NCORES + cc_ * cz2_) + (rr_ - cs2_ * 128)
    in_maps = []
    for c in range(NCORES):
        blo, bhi = c * NBLK, (c + 1) * NBLK
        gidx_c = np.stack([
            _wrap_idx(rowmap[sidx_all[b]]) for b in range(blo, bhi)
        ])
        gidx2_c = np.stack([
            _wrap_idx(rowmap2[sidx_all[b]]) for b in range(blo, bhi)
        ])
        in_maps.append({
            "xT": np.ascontiguousarray(xT[:, c * NPC:(c + 1) * NPC]),
            "w1": w1aug.astype(FP8),
            "w2": np.ascontiguousarray(
                w2aug.reshape(2, 128, 132).astype(BF16)),
            "b1b": b1b, "b2b": b2b, "abb": abb,
            "identin": ident_t,
            "gidx": gidx_c,
            "gidx2": gidx2_c,
            "otp": otp_all[blo:bhi],
            "saT": saT_all[blo:bhi],
            "bhot": bh_all[blo:bhi].astype(BF16),
        })
    return in_maps


def _edge_cfg(inputs):
    ei = np.asarray(inputs["edge_index"])
    x = np.asarray(inputs["x"])
    n = x.shape[0]
    npad = -(-n // (128 * NCORES)) * 128 * NCORES
    npc = npad // NCORES
    dst = np.concatenate([ei[1].astype(np.int64), np.arange(n, dtype=np.int64)])
    cnt = np.bincount(dst // 128, minlength=npad // 128)
    nt1 = int(-(-cnt.max() // 128))
    nt1 += nt1 % 2  # even for DoubleRow pairs
    return {
        "n": n, "npc": npc, "nblk": npc // 128, "nt1": nt1,
        "in_dim": x.shape[1], "hog": 4464 if x.shape[1] == 4527 else None,
        "ng": 64,
    }


def kernel(**inputs):
    global LAST_EXEC_NS
    cfg = _edge_cfg(inputs)
    if cfg["hog"] is None:
        raise ValueError("unexpected input width")
    batch = np.asarray(inputs["batch"]).astype(np.int64)
    Wc1 = np.asarray(inputs["Wc1"], np.float32)
    bc1 = np.asarray(inputs["bc1"], np.float32)
    Wc2 = np.asarray(inputs["Wc2"], np.float32)
    bc2 = np.asarray(inputs["bc2"], np.float32)

    in_maps = prepare_inputs(inputs, cfg)
    nc = build_program(cfg)

    profile = os.environ.get("AGAT_PROFILE", "") == "1"
    if profile:
        _install_ntff_hook()
    res = run_bass_kernel_spmd(
        nc, in_maps, core_ids=list(range(NCORES)), trace=profile,
        tmpdir=os.environ.get("AGAT_PROFILE_DIR") or None,
    )
    if profile:
        LAST_EXEC_NS = res.exec_time_ns

    pooled = np.zeros((cfg["ng"], 128), np.float64)
    for c in range(NCORES):
        pooled += res.results[c]["pout"].astype(np.float64)
    cntg = np.bincount(batch, minlength=cfg["ng"]).astype(np.float64)
    pooled = (pooled / np.maximum(cntg, 1.0)[:, None]).astype(np.float32)
    z = np.maximum(pooled @ Wc1 + bc1, 0.0)
    return (z @ Wc2 + bc2).astype(np.float32)


# revision 45
# speedup vs baseline: 1.0355x; 1.0355x over previous
"""AttentionGAT (2-layer GAT + attention fusion gate + mean-pool + MLP head)
as a Bass/Tile kernel on 8 Trainium2 NeuronCores.

v3: prefetch-pipelined dma_gathers round-robined over the 4 SWDGE queues so
descriptor generation overlaps 4-wide across Q7 core pairs; deeper tile
pools to keep the gather dispatch off the compute-completion critical path;
AllGather chunking with a small tail chunk.

Row layout (haug, 512 fp8 bytes): [0:128]=16*h0 |128|=1.0 [129:257]=16*h1
|257|=1.0; bf16 cols 129,130 (bytes 258:262) = 64*as0, 64*as1; fp8 bytes
262,263 = 64*ad0, 64*ad1.  h2 rows (256B): [0:128]=16*h2 |128|=1.0; bf16
col 65 (bytes 130:132) = 64*as2; fp8 byte 132 = 64*ad2.
"""

import os
import sys
import types

sys.path.insert(0, "/opt/trn_rl_repo")

import numpy as np
import ml_dtypes

import concourse.bass as bass
import concourse.mybir as mybir
import concourse.tile as tile
from concourse import bacc
from concourse import library_config
from concourse.bass_utils import run_bass_kernel_spmd

BF16 = ml_dtypes.bfloat16
FP8 = ml_dtypes.float8_e4m3
NCORES = 8
LAST_EXEC_NS = None  # set when AGAT_PROFILE=1

SH1 = 2.0   # scale on h columns of W1
SH2 = 4.0   # scale on h columns of W2
SA = 64.0   # scale on as/ad projection columns
SG = 32.0   # scale on gate (attn) columns

# AllGather chunk sizes in 128-row tiles per core (small chunks fire early
# during the phase; one mid-size tail chunk).
AG_CHUNKS = [3, 3, 3, 3, 3, 3, 3, 3, 6]
# h2 AllGather chunks: ~1.6MB-out collectives sustain 70-100GB/s while one
# big 7.9MB AG degrades to 22GB/s. All but the last issue late in phase 2a
# (past the last gather dispatch, so their waits block nothing).
H2_CHUNKS = [5, 5, 5, 5, 5, 5]


def _install_ntff_hook():
    try:
        from antenv.axon_hooks import get_axon_ntff_profile_hook  # noqa: F401
        return
    except ImportError:
        pass
    try:
        import antenv
        from trn_agent_boot.trn_boot import _ntff_profile_via_ctypes

        mod = types.ModuleType("antenv.axon_hooks")
        _h = [None]
        mod.set_axon_ntff_profile_hook = lambda h: _h.__setitem__(0, h)
        mod.get_axon_ntff_profile_hook = lambda: _h[0]
        sys.modules["antenv.axon_hooks"] = mod
        antenv.axon_hooks = mod
        mod.set_axon_ntff_profile_hook(
            _ntff_profile_via_ctypes("/opt/axon/libaxon_pjrt.so")
        )
    except Exception:
        pass


def _wrap_idx(a):
    """dma_gather index layout: idx i at [i%16, i//16], tiled to 128 parts."""
    return np.tile(a.reshape(-1, 16).T, (8, 1)).astype(np.int16)


def build_program(cfg):
    IN_DIM, HOG = cfg["in_dim"], cfg["hog"]
    NPC, NBLK, NT1, NG = cfg["npc"], cfg["nblk"], cfg["nt1"], cfg["ng"]
    NPAD = NPC * NCORES
    NPAIR = NT1 // 2
    NC1 = 264          # h0 |z| h1 |z| as0 as1 ad0 ad1 | gate0 gate1
    ROW1, ROW2 = 512, 256
    MG = 3             # node-tiles per phase-1 PSUM group

    # phase-1 K tiling: DoubleRow pairs over the hog region, then singles
    KPAIRS = [(k, k + 256) for k in range(0, HOG - 255, 256)]
    ksing = []
    kp_end = KPAIRS[-1][1] if KPAIRS else 0
    if kp_end < HOG:
        ksing.append((kp_end, HOG))        # hog remainder (psA)
    ksing.append((HOG, IN_DIM))            # cov (psB)

    dt = mybir.dt
    DR = mybir.MatmulPerfMode.DoubleRow
    nc = bacc.Bacc("TRN2", target_bir_lowering=False, debug=False,
                   num_devices=NCORES, num_swdge_queues=4)

    xT = nc.dram_tensor("xT", [IN_DIM, NPC], dt.float8e4, kind="ExternalInput").ap()
    w1 = nc.dram_tensor("w1", [IN_DIM, NC1], dt.float8e4, kind="ExternalInput").ap()
    w2 = nc.dram_tensor("w2", [2, 128, 132], dt.bfloat16, kind="ExternalInput").ap()
    b1b = nc.dram_tensor("b1b", [128, 256], dt.float32, kind="ExternalInput").ap()
    b2b = nc.dram_tensor("b2b", [128, 128], dt.float32, kind="ExternalInput").ap()
    abb = nc.dram_tensor("abb", [128, 2], dt.float32, kind="ExternalInput").ap()
    identin = nc.dram_tensor("identin", [128, 128], dt.bfloat16, kind="ExternalInput").ap()
    gidx = nc.dram_tensor("gidx", [NBLK, 128, NT1 * 8], dt.int16, kind="ExternalInput").ap()
    gidx2 = nc.dram_tensor("gidx2", [NBLK, 128, NT1 * 8], dt.int16, kind="ExternalInput").ap()
    otp = nc.dram_tensor("otp", [NBLK, 128, NT1 * 128], dt.float8e4, kind="ExternalInput").ap()
    saT = nc.dram_tensor("saT", [NBLK, 128, NT1 * 128], dt.float8e4, kind="ExternalInput").ap()
    bhot = nc.dram_tensor("bhot", [NBLK, 128, NG], dt.bfloat16, kind="ExternalInput").ap()
    pout = nc.dram_tensor("pout", [NG, 128], dt.float32, kind="ExternalOutput").ap()

    AOT = mybir.AluOpType
    AFT = mybir.ActivationFunctionType

    # AG chunk boundaries (in tiles)
    ag_starts = []
    s = 0
    for czs in AG_CHUNKS:
        ag_starts.append(s)
        s += czs
    assert s == NBLK

    with tile.TileContext(nc) as tc:
        with (
            tc.tile_pool(name="constp", bufs=1) as constp,
            tc.tile_pool(name="dramp", bufs=1, space="DRAM") as dramp,
        ):
            haug_sh = dramp.tile([NPC, ROW1], dt.float8e4)
            haug = dramp.tile([NPAD, ROW1], dt.float8e4)
            h2_sh = dramp.tile([NPC, ROW2], dt.float8e4)
            h2f = dramp.tile([NPAD, ROW2], dt.float8e4)
            # dummy buffers for a rank-sync barrier AllGather (content unused)
            bar_in = dramp.tile([128, ROW2], dt.float8e4)
            bar_out = dramp.tile([128 * NCORES, ROW2], dt.float8e4)

            nc.gpsimd.load_library(library_config.mlp)
            ident = constp.tile([128, 128], dt.bfloat16)
            nc.sync.dma_start(ident[:], identin[:])
            b1_sb = constp.tile([128, 256], dt.float32)
            nc.sync.dma_start(b1_sb[:], b1b[:])
            b2_sb = constp.tile([128, 128], dt.float32)
            nc.sync.dma_start(b2_sb[:], b2b[:])
            ab_sb = constp.tile([128, 2], dt.float32)
            nc.sync.dma_start(ab_sb[:], abb[:])
            w2_sb = []
            for kk in range(2):
                t = constp.tile([128, 132], dt.bfloat16, tag=f"w2_{kk}",
                                name=f"w2sb{kk}")
                nc.sync.dma_start(t[:], w2[kk])
                w2_sb.append(t)
            # w1 tiles: pairs [128, 2, 264] + singles
            w1p_sb = []
            for k, (k0, k1) in enumerate(KPAIRS):
                t = constp.tile([128, 2, NC1], dt.float8e4, tag=f"w1p_{k}",
                                name=f"w1p{k}")
                nc.sync.dma_start(t[:, 0, :], w1[k0:k0 + 128, :])
                nc.sync.dma_start(t[:, 1, :], w1[k0 + 128:k1, :])
                w1p_sb.append(t)
            w1s_sb = []
            for k, (k0, k1) in enumerate(ksing):
                t = constp.tile([k1 - k0, NC1], dt.float8e4, tag=f"w1s_{k}",
                                name=f"w1s{k}")
                nc.sync.dma_start(t[:], w1[k0:k1, :])
                w1s_sb.append(t)

            # ---------------- phase 1: h_aug for own node shard ------------
            with (
                tc.tile_pool(name="p1x", bufs=1) as p1x,
                tc.tile_pool(name="p1o", bufs=2) as p1o,
                tc.tile_pool(name="p1ps", bufs=1, space="PSUM") as p1ps,
            ):
                NM = NPC // 128
                # resident xT shard in SBUF (one full-row load per k-slice)
                xp = []
                for k, (k0, k1) in enumerate(KPAIRS):
                    t = p1x.tile([128, 2, NPC], dt.float8e4, tag=f"xp{k}",
                                 name=f"xp{k}")
                    nc.sync.dma_start(t[:, 0, :], xT[k0:k0 + 128, :])
                    nc.sync.dma_start(t[:, 1, :], xT[k0 + 128:k1, :])
                    xp.append(t)
                xs = []
                for k, (k0, k1) in enumerate(ksing):
                    t = p1x.tile([k1 - k0, NPC], dt.float8e4, tag=f"xs{k}",
                                 name=f"xs{k}")
                    nc.sync.dma_start(t[:], xT[k0:k1, :])
                    xs.append(t)
                done_chunks = set()
                for g0m in range(0, NM, MG):
                    ms = list(range(g0m, min(g0m + MG, NM)))
                    nms = len(ms)
                    c0 = ms[0] * 128
                    psA = [p1ps.tile([128, NC1], dt.float32, tag=f"A{i}",
                                     name=f"psA{g0m}_{i}") for i in range(nms)]
                    psB = [p1ps.tile([128, NC1], dt.float32, tag=f"B{i}",
                                     name=f"psB{g0m}_{i}") for i in range(nms)]
                    for k, (k0, k1) in enumerate(KPAIRS):
                        for i in range(nms):
                            o0 = c0 + i * 128
                            nc.tensor.matmul(
                                psA[i][:],
                                xp[k][:, :, o0:o0 + 128],
                                w1p_sb[k][:],
                                start=(k == 0), stop=False,
                                perf_mode=DR,
                            )
                    for k, (k0, k1) in enumerate(ksing):
                        cov = (k == len(ksing) - 1)
                        for i in range(nms):
                            o0 = c0 + i * 128
                            nc.tensor.matmul(
                                (psB[i] if cov else psA[i])[:],
                                xs[k][:, o0:o0 + 128],
                                w1s_sb[k][:],
                                start=cov, stop=True,
                            )
                    for i, m in enumerate(ms):
                        lg = p1o.tile([128, 2], dt.float32, tag="lg", name=f"lg{m}")
                        nc.vector.tensor_copy(lg[:], psA[i][:, 262:264])
                        nc.vector.tensor_tensor(lg[:], lg[:],
                                                psB[i][:, 262:264], AOT.add)
                        nc.vector.scalar_tensor_tensor(
                            lg[:], lg[:], 1.0 / SG, ab_sb[:], AOT.mult, AOT.add)
                        mx = p1o.tile([128, 1], dt.float32, tag="mx", name=f"mx{m}")
                        nc.vector.tensor_reduce(mx[:], lg[:],
                                                mybir.AxisListType.X, AOT.max)
                        mxn = p1o.tile([128, 1], dt.float32, tag="mxn", name=f"mxn{m}")
                        nc.vector.tensor_scalar(mxn[:], mx[:], -1.0, None, AOT.mult)
                        em = p1o.tile([128, 2], dt.float32, tag="em", name=f"em{m}")
                        nc.scalar.activation(em[:], lg[:], AFT.Exp, bias=mxn[:, 0:1])
                        sm = p1o.tile([128, 1], dt.float32, tag="sm", name=f"sm{m}")
                        nc.vector.tensor_reduce(sm[:], em[:],
                                                mybir.AxisListType.X, AOT.add)
                        rs = p1o.tile([128, 1], dt.float32, tag="rs", name=f"rs{m}")
                        nc.vector.reciprocal(rs[:], sm[:])
                        gg = p1o.tile([128, 2], dt.float32, tag="gg", name=f"gg{m}")
                        nc.vector.tensor_scalar(gg[:], em[:], rs[:, 0:1], None,
                                                AOT.mult)
                        h1 = p1o.tile([128, 262], dt.float32, tag="h1", name=f"h1{m}")
                        tmb = p1o.tile([128, 262], dt.float32, tag="tmb", name=f"tmb{m}")
                        nc.scalar.mul(h1[:], psA[i][:, 0:262], gg[:, 0:1])
                        nc.scalar.mul(tmb[:], psB[i][:, 0:262], gg[:, 1:2])
                        nc.vector.tensor_tensor(h1[:], h1[:], tmb[:], AOT.add)
                        ha = p1o.tile([128, ROW1], dt.float8e4, tag="ha", name=f"ha{m}")
                        nc.vector.tensor_copy(ha[:, 0:258], h1[:, 0:258])
                        nc.vector.memset(ha[:, 128:129], 1.0)
                        nc.vector.memset(ha[:, 257:258], 1.0)
                        hab = ha[:].bitcast(dt.bfloat16)
                        nc.vector.tensor_copy(hab[:, 129:131], h1[:, 258:260])
                        nc.vector.tensor_copy(ha[:, 262:264], h1[:, 260:262])
                        nc.sync.dma_start(haug_sh[m * 128:(m + 1) * 128, :], ha[:])
                    done_m = ms[-1] + 1
                    for ci, (cs, cz) in enumerate(zip(ag_starts, AG_CHUNKS)):
                        if ci in done_chunks or cs + cz > done_m:
                            continue
                        done_chunks.add(ci)
                        nc.gpsimd.collective_compute(
                            "AllGather", AOT.bypass,
                            replica_groups=[list(range(NCORES))],
                            ins=[haug_sh[cs * 128:(cs + cz) * 128, :]],
                            outs=[haug[cs * 128 * NCORES:
                                       (cs + cz) * 128 * NCORES, :]],
                        )

            # ---------------- phase 2a: layer-1 aggregation -> h2_aug ------
            PF = 4   # gather prefetch depth
            LF = 2   # input-tile (sat/ot/adt) prefetch depth
            with (
                tc.tile_pool(name="p2G", bufs=PF + 3) as p2G,
                tc.tile_pool(name="p2i", bufs=PF + 2) as p2i,
                tc.tile_pool(name="p2in", bufs=LF + 2) as p2in,
                tc.tile_pool(name="p2s", bufs=2) as p2s,
                tc.tile_pool(name="p2o", bufs=2) as p2o,
                tc.tile_pool(name="p2ps", bufs=1, space="PSUM") as p2ps,
            ):
                tiles = {}
                ins2 = {}

                def issue_2a(j):
                    osb = p2i.tile([128, NT1 * 8], dt.int16, tag="osb",
                                   name=f"osb{j}")
                    nc.sync.dma_start(osb[:], gidx[j])
                    G = p2G.tile([128, NT1, ROW1], dt.float8e4, tag="G",
                                 name=f"G{j}")
                    nc.gpsimd.dma_gather(
                        G[:, :, :], haug[:, :], osb[:],
                        NT1 * 128, NT1 * 128, ROW1, single_packet=False,
                        queue_num=j % 4)
                    tiles[j] = (osb, G)

                def load_2a(j):
                    sat = p2in.tile([128, NT1 * 128], dt.float8e4, tag="sat",
                                    name=f"sat{j}")
                    nc.sync.dma_start(sat[:], saT[j])
                    ot = p2in.tile([128, NT1 * 128], dt.float8e4, tag="ot",
                                   name=f"ot{j}")
                    nc.sync.dma_start(ot[:], otp[j])
                    adt = p2in.tile([128, 2], dt.float8e4, tag="adt",
                                    name=f"adt{j}")
                    nc.sync.dma_start(adt[:],
                                      haug_sh[j * 128:(j + 1) * 128, 262:264])
                    ins2[j] = (sat, ot, adt)

                for j in range(PF):
                    issue_2a(j)
                for j in range(LF):
                    load_2a(j)
                for j in range(NBLK):
                    if j + PF < NBLK:
                        issue_2a(j + PF)
                    if j + LF < NBLK:
                        load_2a(j + LF)
                    if j == NBLK - 4:
                        # barrier AG: absorbs cross-rank skew while each
                        # rank's in-flight gathers/compute still run, so the
                        # real h2 chunks below start rank-synchronized
                        nc.gpsimd.collective_compute(
                            "AllGather", AOT.bypass,
                            replica_groups=[list(range(NCORES))],
                            ins=[bar_in[:]], outs=[bar_out[:]],
                        )
                    if j == NBLK - 2:
                        # h2 AG chunks 0..n-2: all later gather issues are
                        # done, so their waits block nothing on the queue
                        cs = 0
                        for cz in H2_CHUNKS[:-1]:
                            nc.gpsimd.collective_compute(
                                "AllGather", AOT.bypass,
                                replica_groups=[list(range(NCORES))],
                                ins=[h2_sh[cs * 128:(cs + cz) * 128, :]],
                                outs=[h2f[cs * 128 * NCORES:
                                          (cs + cz) * 128 * NCORES, :]],
                            )
                            cs += cz
                    osb, G = tiles.pop(j)
                    sat, ot, adt = ins2.pop(j)
                    Gb = G[:].bitcast(dt.bfloat16)
                    adps = p2ps.tile([128, 2 * NT1], dt.float32, tag="adps",
                                     name=f"adps{j}")
                    for t in range(NT1):
                        nc.tensor.matmul(adps[:, 2 * t:2 * t + 2],
                                         ot[:, t * 128:(t + 1) * 128],
                                         adt[:, :], start=True, stop=True)
                    ade = p2s.tile([128, 2 * NT1], dt.float32, tag="ade",
                                   name=f"ade{j}")
                    nc.scalar.mul(ade[:], adps[:], 1.0)
                    es = []
                    for h in (0, 1):
                        z = p2s.tile([128, NT1], dt.float32, tag=f"z{h}",
                                     name=f"z{h}_{j}")
                        nc.vector.tensor_tensor(z[:], Gb[:, :, 129 + h],
                                                ade[:, h::2], AOT.add)
                        nc.vector.scalar_tensor_tensor(
                            z[:], z[:], 0.2, z[:], AOT.mult, AOT.max)
                        e = p2s.tile([128, NT1], dt.float32, tag=f"e{h}",
                                     name=f"e{h}_{j}")
                        nc.scalar.activation(e[:], z[:], AFT.Exp, scale=1.0 / SA)
                        es.append(e)
                    # fold es into G head columns in-place (incl. the 1.0
                    # marker column -> denominator comes out of the matmul);
                    # split into tile-halves so the agg matmuls of the first
                    # half start before the second half's scale completes
                    for h in (0, 1):
                        for t0, t1 in ((0, NT1 // 2), (NT1 // 2, NT1)):
                            gsl = G[:, t0:t1, 129 * h:129 * h + 129]
                            nc.vector.tensor_tensor(
                                gsl, gsl,
                                es[h][:, t0:t1].unsqueeze(2).broadcast_to(
                                    [128, t1 - t0, 129]),
                                AOT.mult)
                    accF = [p2ps.tile([128, 129], dt.float32, tag=f"F{h}",
                                      name=f"F{h}_{j}", bufs=2) for h in (0, 1)]
                    for pi in range(NPAIR):
                        t = 2 * pi
                        sa2 = sat[:, t * 128:(t + 2) * 128].rearrange(
                            "p (k d) -> p k d", k=2)
                        for h in (0, 1):
                            nc.tensor.matmul(
                                accF[h][:], sa2,
                                G[:, t:t + 2, 129 * h:129 * h + 129],
                                start=(pi == 0), stop=(pi == NPAIR - 1),
                                perf_mode=DR)
                    hr = p2o.tile([128, 256], dt.float32, tag="hr", name=f"hr{j}")
                    for h in (0, 1):
                        den = p2o.tile([128, 1], dt.float32, tag=f"den{h}",
                                       name=f"den{h}_{j}")
                        nc.vector.tensor_scalar(den[:], accF[h][:, 128:129],
                                                SH1, SH1 * 1e-6, AOT.mult, AOT.add)
                        rcp = p2o.tile([128, 1], dt.float32, tag=f"rcp{h}",
                                       name=f"rcp{h}_{j}")
                        nc.vector.reciprocal(rcp[:], den[:])
                        nc.scalar.mul(hr[:, h * 128:(h + 1) * 128],
                                      accF[h][:, 0:128], rcp[:, 0:1])
                    nc.vector.tensor_tensor(hr[:], hr[:], b1_sb[:], AOT.add)
                    hrb = p2o.tile([128, 256], dt.bfloat16, tag="hrb", name=f"hrb{j}")
                    nc.scalar.activation(hrb[:], hr[:], AFT.Relu)
                    h2ps = p2ps.tile([128, 132], dt.float32, tag="h2ps",
                                     name=f"h2ps{j}")
                    for kk in range(2):
                        trp = p2ps.tile([128, 128], dt.bfloat16, tag="trp",
                                        name=f"trp{j}_{kk}")
                        nc.tensor.transpose(trp[:],
                                            hrb[:, kk * 128:(kk + 1) * 128],
                                            ident[:])
                        trs = p2s.tile([128, 128], dt.bfloat16, tag="trs",
                                       name=f"trs{j}_{kk}")
                        nc.scalar.mul(trs[:], trp[:], 1.0)
                        nc.tensor.matmul(h2ps[:], trs[:], w2_sb[kk][:],
                                         start=(kk == 0), stop=(kk == 1))
                    h2a = p2o.tile([128, ROW2], dt.float8e4, tag="h2a",
                                   name=f"h2a{j}")
                    nc.scalar.mul(h2a[:, 0:128], h2ps[:, 0:128], 1.0)
                    nc.vector.memset(h2a[:, 128:129], 1.0)
                    h2ab = h2a[:].bitcast(dt.bfloat16)
                    nc.vector.tensor_copy(h2ab[:, 65:66], h2ps[:, 129:130])
                    nc.vector.tensor_copy(h2a[:, 132:133], h2ps[:, 130:131])
                    nc.sync.dma_start(h2_sh[j * 128:(j + 1) * 128, :], h2a[:])
                # h2 AG tail chunk
                cs = sum(H2_CHUNKS[:-1])
                cz = H2_CHUNKS[-1]
                nc.gpsimd.collective_compute(
                    "AllGather", AOT.bypass,
                    replica_groups=[list(range(NCORES))],
                    ins=[h2_sh[cs * 128:(cs + cz) * 128, :]],
                    outs=[h2f[cs * 128 * NCORES:
                              (cs + cz) * 128 * NCORES, :]],
                )

            # ---------------- phase 2b: layer-2 aggregation + pooling ------
            PF2 = 6
            LF2 = 2
            with (
                tc.tile_pool(name="p3G", bufs=PF2 + 3) as p3G,
                tc.tile_pool(name="p3i", bufs=PF2 + 2) as p3i,
                tc.tile_pool(name="p3in", bufs=LF2 + 2) as p3in,
                tc.tile_pool(name="p3s", bufs=2) as p3s,
                tc.tile_pool(name="p3o", bufs=2) as p3o,
                tc.tile_pool(name="p3ps", bufs=1, space="PSUM") as p3ps,
                tc.tile_pool(name="poolps", bufs=1, space="PSUM") as poolps,
            ):
                tiles3 = {}
                ins3 = {}

                def issue_2b(j):
                    osb = p3i.tile([128, NT1 * 8], dt.int16, tag="osb",
                                   name=f"osb3_{j}")
                    nc.sync.dma_start(osb[:], gidx2[j])
                    G = p3G.tile([128, NT1, ROW2], dt.float8e4, tag="G2",
                                 name=f"G2_{j}")
                    nc.gpsimd.dma_gather(
                        G[:, :, :], h2f[:, :], osb[:],
                        NT1 * 128, NT1 * 128, ROW2, single_packet=False,
                        queue_num=j % 4)
                    tiles3[j] = (osb, G)

                def load_2b(j):
                    sat = p3in.tile([128, NT1 * 128], dt.float8e4, tag="sat",
                                    name=f"sat3_{j}")
                    nc.sync.dma_start(sat[:], saT[j])
                    ot = p3in.tile([128, NT1 * 128], dt.float8e4, tag="ot",
                                   name=f"ot3_{j}")
                    nc.sync.dma_start(ot[:], otp[j])
                    adt = p3in.tile([128, 1], dt.float8e4, tag="adt",
                                    name=f"adt3_{j}")
                    nc.sync.dma_start(adt[:],
                                      h2_sh[j * 128:(j + 1) * 128, 132:133])
                    bh = p3in.tile([128, NG], dt.bfloat16, tag="bh", name=f"bh{j}")
                    nc.sync.dma_start(bh[:], bhot[j])
                    ins3[j] = (sat, ot, adt, bh)

                pool_ps = poolps.tile([NG, 128], dt.float32)
                for j in range(PF2):
                    issue_2b(j)
                for j in range(LF2):
                    load_2b(j)
                for j in range(NBLK):
                    if j + PF2 < NBLK:
                        issue_2b(j + PF2)
                    if j + LF2 < NBLK:
                        load_2b(j + LF2)
                    osb, G = tiles3.pop(j)
                    sat, ot, adt, bh = ins3.pop(j)
                    Gb = G[:].bitcast(dt.bfloat16)
                    adps = p3ps.tile([128, NT1], dt.float32, tag="adps",
                                     name=f"adps3_{j}")
                    for t in range(NT1):
                        nc.tensor.matmul(adps[:, t:t + 1],
                                         ot[:, t * 128:(t + 1) * 128],
                                         adt[:, :], start=True, stop=True)
                    ade = p3s.tile([128, NT1], dt.float32, tag="ade",
                                   name=f"ade3_{j}")
                    nc.scalar.mul(ade[:], adps[:], 1.0)
                    z = p3s.tile([128, NT1], dt.float32, tag="z", name=f"z3_{j}")
                    nc.vector.tensor_tensor(z[:], Gb[:, :, 65], ade[:], AOT.add)
                    nc.vector.scalar_tensor_tensor(
                        z[:], z[:], 0.2, z[:], AOT.mult, AOT.max)
                    e = p3s.tile([128, NT1], dt.float32, tag="e", name=f"e3_{j}")
                    nc.scalar.activation(e[:], z[:], AFT.Exp, scale=1.0 / SA)
                    for t0, t1 in ((0, NT1 // 2), (NT1 // 2, NT1)):
                        gsl = G[:, t0:t1, 0:129]
                        nc.vector.tensor_tensor(
                            gsl, gsl,
                            e[:, t0:t1].unsqueeze(2).broadcast_to(
                                [128, t1 - t0, 129]),
                            AOT.mult)
                    accF = p3ps.tile([128, 129], dt.float32, tag="F",
                                     name=f"F3_{j}", bufs=2)
                    for pi in range(NPAIR):
                        t = 2 * pi
                        sa2 = sat[:, t * 128:(t + 2) * 128].rearrange(
                            "p (k d) -> p k d", k=2)
                        nc.tensor.matmul(accF[:], sa2, G[:, t:t + 2, 0:129],
                                         start=(pi == 0), stop=(pi == NPAIR - 1),
                                         perf_mode=DR)
                    den = p3o.tile([128, 1], dt.float32, tag="den", name=f"den3_{j}")
                    nc.vector.tensor_scalar(den[:], accF[:, 128:129], SH2,
                                            SH2 * 1e-6, AOT.mult, AOT.add)
                    rcp = p3o.tile([128, 1], dt.float32, tag="rcp", name=f"rcp3_{j}")
                    nc.vector.reciprocal(rcp[:], den[:])
                    ov = p3o.tile([128, 128], dt.float32, tag="ov", name=f"ov{j}")
                    nc.scalar.mul(ov[:], accF[:, 0:128], rcp[:, 0:1])
                    nc.vector.tensor_tensor(ov[:], ov[:], b2_sb[:], AOT.add)
                    ob = p3o.tile([128, 128], dt.bfloat16, tag="ob", name=f"ob{j}")
                    nc.scalar.activation(ob[:], ov[:], AFT.Relu)
                    nc.tensor.matmul(pool_ps[:], bh[:], ob[:],
                                     start=(j == 0), stop=(j == NBLK - 1))
                pc = p3o.tile([NG, 128], dt.float32, tag="pc")
                nc.vector.tensor_copy(pc[:], pool_ps[:])
                nc.sync.dma_start(pout[:], pc[:])

    nc.compile()
    return nc


def prepare_inputs(inputs, cfg):
    """Host-side sharding/layout. Returns in_maps (one dict per core)."""
    IN_DIM, HOG = cfg["in_dim"], cfg["hog"]
    N, NPC, NBLK, NT1, NG = cfg["n"], cfg["npc"], cfg["nblk"], cfg["nt1"], cfg["ng"]
    NPAD = NPC * NCORES

    x = np.asarray(inputs["x"], np.float32)
    ei = np.asarray(inputs["edge_index"])
    batch = np.asarray(inputs["batch"]).astype(np.int64)
    W1 = np.asarray(inputs["W1"], np.float32)
    a_src1 = np.asarray(inputs["a_src1"], np.float32)
    a_dst1 = np.asarray(inputs["a_dst1"], np.float32)
    W2 = np.asarray(inputs["W2"], np.float32)
    a_src2 = np.asarray(inputs["a_src2"], np.float32)
    a_dst2 = np.asarray(inputs["a_dst2"], np.float32)
    attn_W = np.asarray(inputs["attn_W"], np.float32)
    attn_b = np.asarray(inputs["attn_b"], np.float32)
    b1 = np.asarray(inputs["b1"], np.float32)
    b2 = np.asarray(inputs["b2"], np.float32)

    # augmented weights (scale-folded)
    w1aug = np.zeros((IN_DIM, 264), np.float32)
    w1aug[:, 0:128] = W1[:, 0:128] * SH1
    w1aug[:, 129:257] = W1[:, 128:256] * SH1
    w1aug[:, 258] = W1[:, 0:128] @ a_src1[0] * SA
    w1aug[:, 259] = W1[:, 128:256] @ a_src1[1] * SA
    w1aug[:, 260] = W1[:, 0:128] @ a_dst1[0] * SA
    w1aug[:, 261] = W1[:, 128:256] @ a_dst1[1] * SA
    w1aug[:, 262:264] = attn_W * SG
    w2aug = np.zeros((256, 132), np.float32)
    w2aug[:, 0:128] = W2 * SH2
    w2aug[:, 129] = W2 @ a_src2[0] * SA
    w2aug[:, 130] = W2 @ a_dst2[0] * SA

    xT = np.zeros((IN_DIM, NPAD), FP8)
    xT[:, :N] = np.ascontiguousarray(x.T).astype(FP8)

    # edges sorted by destination, self loops appended
    idt = ei.dtype
    src = np.concatenate([ei[0], np.arange(N, dtype=idt)]).astype(np.int64)
    dst = np.concatenate([ei[1], np.arange(N, dtype=idt)]).astype(np.int64)
    order = np.argsort(dst, kind="stable")
    src_s, dst_s = src[order], dst[order]
    nblk_g = NPAD // 128
    L = NT1 * 128
    cnt = np.bincount(dst_s // 128, minlength=nblk_g)
    assert cnt.max() <= L, (cnt.max(), L)
    offs = np.concatenate([[0], np.cumsum(cnt)])
    sidx_all = np.zeros((nblk_g, L), np.int64)
    dloc_all = np.full((nblk_g, L), -1.0, np.float32)
    for b in range(nblk_g):
        s, e = offs[b], offs[b + 1]
        n = e - s
        sidx_all[b, :n] = src_s[s:e]
        dloc_all[b, :n] = (dst_s[s:e] - 128 * b).astype(np.float32)

    # full one-hot (transposed): otp[b, dl, e] = 1
    otp_all = np.zeros((nblk_g, 128, L), FP8)
    # Sa one-hot in edge-tile layout: saT[b, e%128, (e//128)*128 + dl] = 1
    saT_all = np.zeros((nblk_g, 128, L), FP8)
    eidx = np.arange(L)
    for b in range(nblk_g):
        m = dloc_all[b] >= 0
        dlv = dloc_all[b][m].astype(np.int64)
        ev = eidx[m]
        otp_all[b, dlv, ev] = 1.0
        saT_all[b, ev % 128, (ev // 128) * 128 + dlv] = 1.0

    bh_all = np.zeros((nblk_g, 128, NG), np.float32)
    for b in range(nblk_g):
        base = 128 * b
        hi = min(N - base, 128)
        if hi > 0:
            bh_all[b, np.arange(hi), batch[base:base + hi]] = 1.0

    ident_t = np.eye(128, dtype=np.float32).astype(BF16)
    b1b = np.tile(b1[None, :], (128, 1)).astype(np.float32)
    b2b = np.tile(b2[None, :], (128, 1)).astype(np.float32)
    abb = np.tile(attn_b[None, :], (128, 1)).astype(np.float32)

    # chunk-major AllGather layout with chunk sizes AG_CHUNKS (in tiles):
    # node n (core c, local row r, local tile m=r//128, chunk ci) ->
    #   row 128*(cs*8 + c*cz) + (r - cs*128)
    tile_chunk = np.zeros(NBLK, np.int64)
    chunk_start = np.zeros(len(AG_CHUNKS), np.int64)
    s = 0
    for ci, cz in enumerate(AG_CHUNKS):
        chunk_start[ci] = s
        tile_chunk[s:s + cz] = ci
        s += cz
    n_arr = np.arange(NPAD, dtype=np.int64)
    cc_, rr_ = n_arr // NPC, n_arr % NPC
    mm_ = rr_ // 128
    ci_ = tile_chunk[mm_]
    cs_ = chunk_start[ci_]
    cz_ = np.asarray(AG_CHUNKS, np.int64)[ci_]
    rowmap = 128 * (cs_ * NCORES + cc_ * cz_) + (rr_ - cs_ * 128)
    # h2f chunk-major layout per H2_CHUNKS
    tile_chunk2 = np.zeros(NBLK, np.int64)
    chunk_start2 = np.zeros(len(H2_CHUNKS), np.int64)
    s = 0
    for ci, cz in enumerate(H2_CHUNKS):
        chunk_start2[ci] = s
        tile_chunk2[s:s + cz] = ci
        s += cz
    ci2_ = tile_chunk2[mm_]
    cs2_ = chunk_start2[ci2_]
    cz2_ = np.asarray(H2_CHUNKS, np.int64)[ci2_]
    rowmap2 = 128 * (cs2_ * NCORES + cc_ * cz2_) + (rr_ - cs2_ * 128)
    in_maps = []
    for c in range(NCORES):
        blo, bhi = c * NBLK, (c + 1) * NBLK
        gidx_c = np.stack([
            _wrap_idx(rowmap[sidx_all[b]]) for b in range(blo, bhi)
        ])
        gidx2_c = np.stack([
            _wrap_idx(rowmap2[sidx_all[b]]) for b in range(blo, bhi)
        ])
        in_maps.append({
            "xT": np.ascontiguousarray(xT[:, c * NPC:(c + 1) * NPC]),
            "w1": w1aug.astype(FP8),
            "w2": np.ascontiguousarray(
                w2aug.reshape(2, 128, 132).astype(BF16)),
            "b1b": b1b, "b2b": b2b, "abb": abb,
            "identin": ident_t,
            "gidx": gidx_c,
            "gidx2": gidx2_c,
            "otp": otp_all[blo:bhi],
            "saT": saT_all[blo:bhi],
            "bhot": bh_all[blo:bhi].astype(BF16),
        })
    return in_maps


def _edge_cfg(inputs):
    ei = np.asarray(inputs["edge_index"])
    x = np.asarray(inputs["x"])
    n = x.shape[0]
    npad = -(-n // (128 * NCORES)) * 128 * NCORES
    npc = npad // NCORES
    dst = np.concatenate([ei[1].astype(np.int64), np.arange(n, dtype=np.int64)])
    cnt = np.bincount(dst // 128, minlength=npad // 128)
    nt1 = int(-(-cnt.max() // 128))
    nt1 += nt1 % 2  # even for DoubleRow pairs
    return {
        "n": n, "npc": npc, "nblk": npc // 128, "nt1": nt1,
        "in_dim": x.shape[1], "hog": 4464 if x.shape[1] == 4527 else None,
        "ng": 64,
    }


def kernel(**inputs):
    global LAST_EXEC_NS
    cfg = _edge_cfg(inputs)
    if cfg["hog"] is None:
        raise ValueError("unexpected input width")
    batch = np.asarray(inputs["batch"]).astype(np.int64)
    Wc1 = np.asarray(inputs["Wc1"], np.float32)
    bc1 = np.asarray(inputs["bc1"], np.float32)
    Wc2 = np.asarray(inputs["Wc2"], np.float32)
    bc2 = np.asarray(inputs["bc2"], np.float32)

    in_maps = prepare_inputs(inputs, cfg)
    nc = build_program(cfg)

    profile = os.environ.get("AGAT_PROFILE", "") == "1"
    if profile:
        _install_ntff_hook()
    res = run_bass_kernel_spmd(
        nc, in_maps, core_ids=list(range(NCORES)), trace=profile,
        tmpdir=os.environ.get("AGAT_PROFILE_DIR") or None,
    )
    if profile:
        LAST_EXEC_NS = res.exec_time_ns

    pooled = np.zeros((cfg["ng"], 128), np.float64)
    for c in range(NCORES):
        pooled += res.results[c]["pout"].astype(np.float64)
    cntg = np.bincount(batch, minlength=cfg["ng"]).astype(np.float64)
    pooled = (pooled / np.maximum(cntg, 1.0)[:, None]).astype(np.float32)
    z = np.maximum(pooled @ Wc1 + bc1, 0.0)
    return (z @ Wc2 + bc2).astype(np.float32)


# revision 49
# speedup vs baseline: 1.0574x; 1.0212x over previous
"""AttentionGAT (2-layer GAT + attention fusion gate + mean-pool + MLP head)
as a Bass/Tile kernel on 8 Trainium2 NeuronCores.

v3: prefetch-pipelined dma_gathers round-robined over the 4 SWDGE queues so
descriptor generation overlaps 4-wide across Q7 core pairs; deeper tile
pools to keep the gather dispatch off the compute-completion critical path;
AllGather chunking with a small tail chunk.

Row layout (haug, 512 fp8 bytes): [0:128]=16*h0 |128|=1.0 [129:257]=16*h1
|257|=1.0; bf16 cols 129,130 (bytes 258:262) = 64*as0, 64*as1; fp8 bytes
262,263 = 64*ad0, 64*ad1.  h2 rows (256B): [0:128]=16*h2 |128|=1.0; bf16
col 65 (bytes 130:132) = 64*as2; fp8 byte 132 = 64*ad2.
"""

import os
import sys
import types

sys.path.insert(0, "/opt/trn_rl_repo")

import numpy as np
import ml_dtypes

import concourse.bass as bass
import concourse.mybir as mybir
import concourse.tile as tile
from concourse import bacc
from concourse import library_config
from concourse.bass_utils import run_bass_kernel_spmd

BF16 = ml_dtypes.bfloat16
FP8 = ml_dtypes.float8_e4m3
NCORES = 8
LAST_EXEC_NS = None  # set when AGAT_PROFILE=1

SH1 = 2.0   # scale on h columns of W1
SH2 = 4.0   # scale on h columns of W2
SA = 64.0   # scale on as/ad projection columns
SG = 32.0   # scale on gate (attn) columns

# AllGather chunk sizes in 128-row tiles per core (small chunks fire early
# during the phase; one mid-size tail chunk).
AG_CHUNKS = [3, 3, 3, 3, 3, 3, 3, 3, 6]
# h2 AllGather chunks: ~1.6MB-out collectives sustain 70-100GB/s while one
# big 7.9MB AG degrades to 22GB/s. All but the last issue late in phase 2a
# (past the last gather dispatch, so their waits block nothing).
H2_CHUNKS = [5, 5, 5, 5, 5, 5]


def _install_ntff_hook():
    try:
        from antenv.axon_hooks import get_axon_ntff_profile_hook  # noqa: F401
        return
    except ImportError:
        pass
    try:
        import antenv
        from trn_agent_boot.trn_boot import _ntff_profile_via_ctypes

        mod = types.ModuleType("antenv.axon_hooks")
        _h = [None]
        mod.set_axon_ntff_profile_hook = lambda h: _h.__setitem__(0, h)
        mod.get_axon_ntff_profile_hook = lambda: _h[0]
        sys.modules["antenv.axon_hooks"] = mod
        antenv.axon_hooks = mod
        mod.set_axon_ntff_profile_hook(
            _ntff_profile_via_ctypes("/opt/axon/libaxon_pjrt.so")
        )
    except Exception:
        pass


def _wrap_idx(a):
    """dma_gather index layout: idx i at [i%16, i//16], tiled to 128 parts."""
    return np.tile(a.reshape(-1, 16).T, (8, 1)).astype(np.int16)


def build_program(cfg):
    IN_DIM, HOG = cfg["in_dim"], cfg["hog"]
    NPC, NBLK, NT1, NG = cfg["npc"], cfg["nblk"], cfg["nt1"], cfg["ng"]
    NPAD = NPC * NCORES
    NPAIR = NT1 // 2
    NC1 = 264          # h0 |z| h1 |z| as0 as1 ad0 ad1 | gate0 gate1
    ROW1, ROW2 = 512, 256
    MG = 3             # node-tiles per phase-1 PSUM group

    # phase-1 K tiling: DoubleRow pairs over the hog region, then singles
    KPAIRS = [(k, k + 256) for k in range(0, HOG - 255, 256)]
    ksing = []
    kp_end = KPAIRS[-1][1] if KPAIRS else 0
    if kp_end < HOG:
        ksing.append((kp_end, HOG))        # hog remainder (psA)
    ksing.append((HOG, IN_DIM))            # cov (psB)

    dt = mybir.dt
    DR = mybir.MatmulPerfMode.DoubleRow
    nc = bacc.Bacc("TRN2", target_bir_lowering=False, debug=False,
                   num_devices=NCORES, num_swdge_queues=4)

    xT = nc.dram_tensor("xT", [IN_DIM, NPC], dt.float8e4, kind="ExternalInput").ap()
    w1 = nc.dram_tensor("w1", [IN_DIM, NC1], dt.float8e4, kind="ExternalInput").ap()
    w2 = nc.dram_tensor("w2", [2, 128, 132], dt.bfloat16, kind="ExternalInput").ap()
    b1b = nc.dram_tensor("b1b", [128, 256], dt.float32, kind="ExternalInput").ap()
    b2b = nc.dram_tensor("b2b", [128, 128], dt.float32, kind="ExternalInput").ap()
    abb = nc.dram_tensor("abb", [128, 2], dt.float32, kind="ExternalInput").ap()
    identin = nc.dram_tensor("identin", [128, 128], dt.bfloat16, kind="ExternalInput").ap()
    gidx = nc.dram_tensor("gidx", [NBLK, 128, NT1 * 8], dt.int16, kind="ExternalInput").ap()
    gidx2 = nc.dram_tensor("gidx2", [NBLK, 128, NT1 * 8], dt.int16, kind="ExternalInput").ap()
    otp = nc.dram_tensor("otp", [NBLK, 128, NT1 * 128], dt.float8e4, kind="ExternalInput").ap()
    saT = nc.dram_tensor("saT", [NBLK, 128, NT1 * 128], dt.float8e4, kind="ExternalInput").ap()
    bhot = nc.dram_tensor("bhot", [NBLK, 128, NG], dt.bfloat16, kind="ExternalInput").ap()
    pout = nc.dram_tensor("pout", [NG, 128], dt.float32, kind="ExternalOutput").ap()

    AOT = mybir.AluOpType
    AFT = mybir.ActivationFunctionType

    # AG chunk boundaries (in tiles)
    ag_starts = []
    s = 0
    for czs in AG_CHUNKS:
        ag_starts.append(s)
        s += czs
    assert s == NBLK

    with tile.TileContext(nc) as tc:
        with (
            tc.tile_pool(name="constp", bufs=1) as constp,
            tc.tile_pool(name="dramp", bufs=1, space="DRAM") as dramp,
        ):
            haug_sh = dramp.tile([NPC, ROW1], dt.float8e4)
            haug = dramp.tile([NPAD, ROW1], dt.float8e4)
            h2_sh = dramp.tile([NPC, ROW2], dt.float8e4)
            h2f = dramp.tile([NPAD, ROW2], dt.float8e4)
            # dummy buffers for a rank-sync barrier AllGather (content unused)
            bar_in = dramp.tile([128, ROW2], dt.float8e4)
            bar_out = dramp.tile([128 * NCORES, ROW2], dt.float8e4)

            nc.gpsimd.load_library(library_config.mlp)
            ident = constp.tile([128, 128], dt.bfloat16)
            nc.sync.dma_start(ident[:], identin[:])
            b1_sb = constp.tile([128, 256], dt.float32)
            nc.sync.dma_start(b1_sb[:], b1b[:])
            b2_sb = constp.tile([128, 128], dt.float32)
            nc.sync.dma_start(b2_sb[:], b2b[:])
            ab_sb = constp.tile([128, 2], dt.float32)
            nc.sync.dma_start(ab_sb[:], abb[:])
            w2_sb = []
            for kk in range(2):
                t = constp.tile([128, 132], dt.bfloat16, tag=f"w2_{kk}",
                                name=f"w2sb{kk}")
                nc.sync.dma_start(t[:], w2[kk])
                w2_sb.append(t)
            # w1 tiles: pairs [128, 2, 264] + singles
            w1p_sb = []
            for k, (k0, k1) in enumerate(KPAIRS):
                t = constp.tile([128, 2, NC1], dt.float8e4, tag=f"w1p_{k}",
                                name=f"w1p{k}")
                nc.sync.dma_start(t[:, 0, :], w1[k0:k0 + 128, :])
                nc.sync.dma_start(t[:, 1, :], w1[k0 + 128:k1, :])
                w1p_sb.append(t)
            w1s_sb = []
            for k, (k0, k1) in enumerate(ksing):
                t = constp.tile([k1 - k0, NC1], dt.float8e4, tag=f"w1s_{k}",
                                name=f"w1s{k}")
                nc.sync.dma_start(t[:], w1[k0:k1, :])
                w1s_sb.append(t)

            # ---------------- phase 1: h_aug for own node shard ------------
            with (
                tc.tile_pool(name="p1x", bufs=1) as p1x,
                tc.tile_pool(name="p1o", bufs=2) as p1o,
                tc.tile_pool(name="p1ps", bufs=1, space="PSUM") as p1ps,
            ):
                NM = NPC // 128
                # resident xT shard in SBUF (one full-row load per k-slice)
                xp = []
                for k, (k0, k1) in enumerate(KPAIRS):
                    t = p1x.tile([128, 2, NPC], dt.float8e4, tag=f"xp{k}",
                                 name=f"xp{k}")
                    nc.sync.dma_start(t[:, 0, :], xT[k0:k0 + 128, :])
                    nc.sync.dma_start(t[:, 1, :], xT[k0 + 128:k1, :])
                    xp.append(t)
                xs = []
                for k, (k0, k1) in enumerate(ksing):
                    t = p1x.tile([k1 - k0, NPC], dt.float8e4, tag=f"xs{k}",
                                 name=f"xs{k}")
                    nc.sync.dma_start(t[:], xT[k0:k1, :])
                    xs.append(t)
                done_chunks = set()
                for g0m in range(0, NM, MG):
                    ms = list(range(g0m, min(g0m + MG, NM)))
                    nms = len(ms)
                    c0 = ms[0] * 128
                    psA = [p1ps.tile([128, NC1], dt.float32, tag=f"A{i}",
                                     name=f"psA{g0m}_{i}") for i in range(nms)]
                    psB = [p1ps.tile([128, NC1], dt.float32, tag=f"B{i}",
                                     name=f"psB{g0m}_{i}") for i in range(nms)]
                    for k, (k0, k1) in enumerate(KPAIRS):
                        for i in range(nms):
                            o0 = c0 + i * 128
                            nc.tensor.matmul(
                                psA[i][:],
                                xp[k][:, :, o0:o0 + 128],
                                w1p_sb[k][:],
                                start=(k == 0), stop=False,
                                perf_mode=DR,
                            )
                    for k, (k0, k1) in enumerate(ksing):
                        cov = (k == len(ksing) - 1)
                        for i in range(nms):
                            o0 = c0 + i * 128
                            nc.tensor.matmul(
                                (psB[i] if cov else psA[i])[:],
                                xs[k][:, o0:o0 + 128],
                                w1s_sb[k][:],
                                start=cov, stop=True,
                            )
                    for i, m in enumerate(ms):
                        lg = p1o.tile([128, 2], dt.float32, tag="lg", name=f"lg{m}")
                        nc.vector.tensor_copy(lg[:], psA[i][:, 262:264])
                        nc.vector.tensor_tensor(lg[:], lg[:],
                                                psB[i][:, 262:264], AOT.add)
                        nc.vector.scalar_tensor_tensor(
                            lg[:], lg[:], 1.0 / SG, ab_sb[:], AOT.mult, AOT.add)
                        mx = p1o.tile([128, 1], dt.float32, tag="mx", name=f"mx{m}")
                        nc.vector.tensor_reduce(mx[:], lg[:],
                                                mybir.AxisListType.X, AOT.max)
                        mxn = p1o.tile([128, 1], dt.float32, tag="mxn", name=f"mxn{m}")
                        nc.vector.tensor_scalar(mxn[:], mx[:], -1.0, None, AOT.mult)
                        em = p1o.tile([128, 2], dt.float32, tag="em", name=f"em{m}")
                        nc.scalar.activation(em[:], lg[:], AFT.Exp, bias=mxn[:, 0:1])
                        sm = p1o.tile([128, 1], dt.float32, tag="sm", name=f"sm{m}")
                        nc.vector.tensor_reduce(sm[:], em[:],
                                                mybir.AxisListType.X, AOT.add)
                        rs = p1o.tile([128, 1], dt.float32, tag="rs", name=f"rs{m}")
                        nc.vector.reciprocal(rs[:], sm[:])
                        gg = p1o.tile([128, 2], dt.float32, tag="gg", name=f"gg{m}")
                        nc.vector.tensor_scalar(gg[:], em[:], rs[:, 0:1], None,
                                                AOT.mult)
                        h1 = p1o.tile([128, 262], dt.float32, tag="h1", name=f"h1{m}")
                        tmb = p1o.tile([128, 262], dt.float32, tag="tmb", name=f"tmb{m}")
                        nc.scalar.mul(h1[:], psA[i][:, 0:262], gg[:, 0:1])
                        nc.scalar.mul(tmb[:], psB[i][:, 0:262], gg[:, 1:2])
                        nc.vector.tensor_tensor(h1[:], h1[:], tmb[:], AOT.add)
                        ha = p1o.tile([128, ROW1], dt.float8e4, tag="ha", name=f"ha{m}")
                        nc.vector.tensor_copy(ha[:, 0:258], h1[:, 0:258])
                        nc.vector.memset(ha[:, 128:129], 1.0)
                        nc.vector.memset(ha[:, 257:258], 1.0)
                        hab = ha[:].bitcast(dt.bfloat16)
                        nc.vector.tensor_copy(hab[:, 129:131], h1[:, 258:260])
                        nc.vector.tensor_copy(ha[:, 262:264], h1[:, 260:262])
                        nc.sync.dma_start(haug_sh[m * 128:(m + 1) * 128, :], ha[:])
                    done_m = ms[-1] + 1
                    for ci, (cs, cz) in enumerate(zip(ag_starts, AG_CHUNKS)):
                        if ci in done_chunks or cs + cz > done_m:
                            continue
                        done_chunks.add(ci)
                        nc.gpsimd.collective_compute(
                            "AllGather", AOT.bypass,
                            replica_groups=[list(range(NCORES))],
                            ins=[haug_sh[cs * 128:(cs + cz) * 128, :]],
                            outs=[haug[cs * 128 * NCORES:
                                       (cs + cz) * 128 * NCORES, :]],
                        )

            # ---------------- phase 2a: layer-1 aggregation -> h2_aug ------
            PF = 4   # gather prefetch depth
            LF = 2   # input-tile (sat/ot/adt) prefetch depth
            with (
                tc.tile_pool(name="p2G", bufs=PF + 3) as p2G,
                tc.tile_pool(name="p2i", bufs=PF + 2) as p2i,
                tc.tile_pool(name="p2in", bufs=LF + 2) as p2in,
                tc.tile_pool(name="p2s", bufs=2) as p2s,
                tc.tile_pool(name="p2o", bufs=2) as p2o,
                tc.tile_pool(name="p2ps", bufs=1, space="PSUM") as p2ps,
            ):
                tiles = {}
                ins2 = {}

                HNT = NT1 // 2

                def issue_2a(j):
                    # two half-gathers on distinct queues: their descgens run
                    # concurrently on two Q7 pairs, halving per-block latency
                    osb = p2i.tile([128, NT1 * 8], dt.int16, tag="osb",
                                   name=f"osb{j}")
                    nc.sync.dma_start(osb[:], gidx[j])
                    Gh = []
                    for hf in (0, 1):
                        G = p2G.tile([128, HNT, ROW1], dt.float8e4,
                                     tag=f"G{hf}", name=f"G{hf}_{j}")
                        nc.gpsimd.dma_gather(
                            G[:, :, :], haug[:, :],
                            osb[:, hf * (HNT * 8):(hf + 1) * (HNT * 8)],
                            HNT * 128, HNT * 128, ROW1, single_packet=False,
                            queue_num=(2 * j + hf) % 4)
                        Gh.append(G)
                    tiles[j] = (osb, Gh)

                def load_2a(j):
                    sat = p2in.tile([128, NT1 * 128], dt.float8e4, tag="sat",
                                    name=f"sat{j}")
                    nc.sync.dma_start(sat[:], saT[j])
                    ot = p2in.tile([128, NT1 * 128], dt.float8e4, tag="ot",
                                   name=f"ot{j}")
                    nc.sync.dma_start(ot[:], otp[j])
                    adt = p2in.tile([128, 2], dt.float8e4, tag="adt",
                                    name=f"adt{j}")
                    nc.sync.dma_start(adt[:],
                                      haug_sh[j * 128:(j + 1) * 128, 262:264])
                    ins2[j] = (sat, ot, adt)

                for j in range(PF):
                    issue_2a(j)
                for j in range(LF):
                    load_2a(j)
                for j in range(NBLK):
                    if j + PF < NBLK:
                        issue_2a(j + PF)
                    if j + LF < NBLK:
                        load_2a(j + LF)
                    if j == NBLK - 4:
                        # barrier AG: absorbs cross-rank skew while each
                        # rank's in-flight gathers/compute still run, so the
                        # real h2 chunks below start rank-synchronized
                        nc.gpsimd.collective_compute(
                            "AllGather", AOT.bypass,
                            replica_groups=[list(range(NCORES))],
                            ins=[bar_in[:]], outs=[bar_out[:]],
                        )
                    if j == NBLK - 2:
                        # h2 AG chunks 0..n-2: all later gather issues are
                        # done, so their waits block nothing on the queue
                        cs = 0
                        for cz in H2_CHUNKS[:-1]:
                            nc.gpsimd.collective_compute(
                                "AllGather", AOT.bypass,
                                replica_groups=[list(range(NCORES))],
                                ins=[h2_sh[cs * 128:(cs + cz) * 128, :]],
                                outs=[h2f[cs * 128 * NCORES:
                                          (cs + cz) * 128 * NCORES, :]],
                            )
                            cs += cz
                    osb, (G0, G1) = tiles.pop(j)
                    sat, ot, adt = ins2.pop(j)
                    adps = p2ps.tile([128, 2 * NT1], dt.float32, tag="adps",
                                     name=f"adps{j}")
                    for t in range(NT1):
                        nc.tensor.matmul(adps[:, 2 * t:2 * t + 2],
                                         ot[:, t * 128:(t + 1) * 128],
                                         adt[:, :], start=True, stop=True)
                    ade = p2s.tile([128, 2 * NT1], dt.float32, tag="ade",
                                   name=f"ade{j}")
                    nc.scalar.mul(ade[:], adps[:], 1.0)
                    es = []
                    for h in (0, 1):
                        z = p2s.tile([128, NT1], dt.float32, tag=f"z{h}",
                                     name=f"z{h}_{j}")
                        for hf, G in ((0, G0), (1, G1)):
                            Gb = G[:].bitcast(dt.bfloat16)
                            nc.vector.tensor_tensor(
                                z[:, hf * HNT:(hf + 1) * HNT],
                                Gb[:, :, 129 + h],
                                ade[:, 2 * hf * HNT + h:
                                     2 * (hf + 1) * HNT:2], AOT.add)
                        nc.vector.scalar_tensor_tensor(
                            z[:], z[:], 0.2, z[:], AOT.mult, AOT.max)
                        e = p2s.tile([128, NT1], dt.float32, tag=f"e{h}",
                                     name=f"e{h}_{j}")
                        nc.scalar.activation(e[:], z[:], AFT.Exp, scale=1.0 / SA)
                        es.append(e)
                    # fold es into G head columns in-place (incl. the 1.0
                    # marker column -> denominator comes out of the matmul)
                    for h in (0, 1):
                        for hf, G in ((0, G0), (1, G1)):
                            gsl = G[:, :, 129 * h:129 * h + 129]
                            nc.vector.tensor_tensor(
                                gsl, gsl,
                                es[h][:, hf * HNT:(hf + 1) * HNT]
                                .unsqueeze(2).broadcast_to([128, HNT, 129]),
                                AOT.mult)
                    accF = [p2ps.tile([128, 129], dt.float32, tag=f"F{h}",
                                      name=f"F{h}_{j}", bufs=2) for h in (0, 1)]
                    for pi in range(NPAIR):
                        t = 2 * pi
                        hf = 0 if pi < NPAIR // 2 else 1
                        G = G0 if hf == 0 else G1
                        tl = t - hf * HNT
                        sa2 = sat[:, t * 128:(t + 2) * 128].rearrange(
                            "p (k d) -> p k d", k=2)
                        for h in (0, 1):
                            nc.tensor.matmul(
                                accF[h][:], sa2,
                                G[:, tl:tl + 2, 129 * h:129 * h + 129],
                                start=(pi == 0), stop=(pi == NPAIR - 1),
                                perf_mode=DR)
                    hr = p2o.tile([128, 256], dt.float32, tag="hr", name=f"hr{j}")
                    for h in (0, 1):
                        den = p2o.tile([128, 1], dt.float32, tag=f"den{h}",
                                       name=f"den{h}_{j}")
                        nc.vector.tensor_scalar(den[:], accF[h][:, 128:129],
                                                SH1, SH1 * 1e-6, AOT.mult, AOT.add)
                        rcp = p2o.tile([128, 1], dt.float32, tag=f"rcp{h}",
                                       name=f"rcp{h}_{j}")
                        nc.vector.reciprocal(rcp[:], den[:])
                        nc.scalar.mul(hr[:, h * 128:(h + 1) * 128],
                                      accF[h][:, 0:128], rcp[:, 0:1])
                    nc.vector.tensor_tensor(hr[:], hr[:], b1_sb[:], AOT.add)
                    hrb = p2o.tile([128, 256], dt.bfloat16, tag="hrb", name=f"hrb{j}")
                    nc.scalar.activation(hrb[:], hr[:], AFT.Relu)
                    h2ps = p2ps.tile([128, 132], dt.float32, tag="h2ps",
                                     name=f"h2ps{j}")
                    for kk in range(2):
                        trp = p2ps.tile([128, 128], dt.bfloat16, tag="trp",
                                        name=f"trp{j}_{kk}")
                        nc.tensor.transpose(trp[:],
                                            hrb[:, kk * 128:(kk + 1) * 128],
                                            ident[:])
                        trs = p2s.tile([128, 128], dt.bfloat16, tag="trs",
                                       name=f"trs{j}_{kk}")
                        nc.scalar.mul(trs[:], trp[:], 1.0)
                        nc.tensor.matmul(h2ps[:], trs[:], w2_sb[kk][:],
                                         start=(kk == 0), stop=(kk == 1))
                    h2a = p2o.tile([128, ROW2], dt.float8e4, tag="h2a",
                                   name=f"h2a{j}")
                    nc.scalar.mul(h2a[:, 0:128], h2ps[:, 0:128], 1.0)
                    nc.vector.memset(h2a[:, 128:129], 1.0)
                    h2ab = h2a[:].bitcast(dt.bfloat16)
                    nc.vector.tensor_copy(h2ab[:, 65:66], h2ps[:, 129:130])
                    nc.vector.tensor_copy(h2a[:, 132:133], h2ps[:, 130:131])
                    nc.sync.dma_start(h2_sh[j * 128:(j + 1) * 128, :], h2a[:])
                # h2 AG tail chunk
                cs = sum(H2_CHUNKS[:-1])
                cz = H2_CHUNKS[-1]
                nc.gpsimd.collective_compute(
                    "AllGather", AOT.bypass,
                    replica_groups=[list(range(NCORES))],
                    ins=[h2_sh[cs * 128:(cs + cz) * 128, :]],
                    outs=[h2f[cs * 128 * NCORES:
                              (cs + cz) * 128 * NCORES, :]],
                )

            # ---------------- phase 2b: layer-2 aggregation + pooling ------
            PF2 = 6
            LF2 = 2
            with (
                tc.tile_pool(name="p3G", bufs=PF2 + 3) as p3G,
                tc.tile_pool(name="p3i", bufs=PF2 + 2) as p3i,
                tc.tile_pool(name="p3in", bufs=LF2 + 2) as p3in,
                tc.tile_pool(name="p3s", bufs=2) as p3s,
                tc.tile_pool(name="p3o", bufs=2) as p3o,
                tc.tile_pool(name="p3ps", bufs=1, space="PSUM") as p3ps,
                tc.tile_pool(name="poolps", bufs=1, space="PSUM") as poolps,
            ):
                tiles3 = {}
                ins3 = {}

                HNT = NT1 // 2

                def issue_2b(j):
                    osb = p3i.tile([128, NT1 * 8], dt.int16, tag="osb",
                                   name=f"osb3_{j}")
                    nc.sync.dma_start(osb[:], gidx2[j])
                    Gh = []
                    for hf in (0, 1):
                        G = p3G.tile([128, HNT, ROW2], dt.float8e4,
                                     tag=f"G2{hf}", name=f"G2{hf}_{j}")
                        nc.gpsimd.dma_gather(
                            G[:, :, :], h2f[:, :],
                            osb[:, hf * (HNT * 8):(hf + 1) * (HNT * 8)],
                            HNT * 128, HNT * 128, ROW2, single_packet=False,
                            queue_num=(2 * j + hf) % 4)
                        Gh.append(G)
                    tiles3[j] = (osb, Gh)

                def load_2b(j):
                    sat = p3in.tile([128, NT1 * 128], dt.float8e4, tag="sat",
                                    name=f"sat3_{j}")
                    nc.sync.dma_start(sat[:], saT[j])
                    ot = p3in.tile([128, NT1 * 128], dt.float8e4, tag="ot",
                                   name=f"ot3_{j}")
                    nc.sync.dma_start(ot[:], otp[j])
                    adt = p3in.tile([128, 1], dt.float8e4, tag="adt",
                                    name=f"adt3_{j}")
                    nc.sync.dma_start(adt[:],
                                      h2_sh[j * 128:(j + 1) * 128, 132:133])
                    bh = p3in.tile([128, NG], dt.bfloat16, tag="bh", name=f"bh{j}")
                    nc.sync.dma_start(bh[:], bhot[j])
                    ins3[j] = (sat, ot, adt, bh)

                pool_ps = poolps.tile([NG, 128], dt.float32)
                for j in range(PF2):
                    issue_2b(j)
                for j in range(LF2):
                    load_2b(j)
                for j in range(NBLK):
                    if j + PF2 < NBLK:
                        issue_2b(j + PF2)
                    if j + LF2 < NBLK:
                        load_2b(j + LF2)
                    osb, (G0, G1) = tiles3.pop(j)
                    sat, ot, adt, bh = ins3.pop(j)
                    adps = p3ps.tile([128, NT1], dt.float32, tag="adps",
                                     name=f"adps3_{j}")
                    for t in range(NT1):
                        nc.tensor.matmul(adps[:, t:t + 1],
                                         ot[:, t * 128:(t + 1) * 128],
                                         adt[:, :], start=True, stop=True)
                    ade = p3s.tile([128, NT1], dt.float32, tag="ade",
                                   name=f"ade3_{j}")
                    nc.scalar.mul(ade[:], adps[:], 1.0)
                    z = p3s.tile([128, NT1], dt.float32, tag="z", name=f"z3_{j}")
                    for hf, G in ((0, G0), (1, G1)):
                        Gb = G[:].bitcast(dt.bfloat16)
                        nc.vector.tensor_tensor(
                            z[:, hf * HNT:(hf + 1) * HNT], Gb[:, :, 65],
                            ade[:, hf * HNT:(hf + 1) * HNT], AOT.add)
                    nc.vector.scalar_tensor_tensor(
                        z[:], z[:], 0.2, z[:], AOT.mult, AOT.max)
                    e = p3s.tile([128, NT1], dt.float32, tag="e", name=f"e3_{j}")
                    nc.scalar.activation(e[:], z[:], AFT.Exp, scale=1.0 / SA)
                    for hf, G in ((0, G0), (1, G1)):
                        gsl = G[:, :, 0:129]
                        nc.vector.tensor_tensor(
                            gsl, gsl,
                            e[:, hf * HNT:(hf + 1) * HNT]
                            .unsqueeze(2).broadcast_to([128, HNT, 129]),
                            AOT.mult)
                    accF = p3ps.tile([128, 129], dt.float32, tag="F",
                                     name=f"F3_{j}", bufs=2)
                    for pi in range(NPAIR):
                        t = 2 * pi
                        hf = 0 if pi < NPAIR // 2 else 1
                        G = G0 if hf == 0 else G1
                        tl = t - hf * HNT
                        sa2 = sat[:, t * 128:(t + 2) * 128].rearrange(
                            "p (k d) -> p k d", k=2)
                        nc.tensor.matmul(accF[:], sa2, G[:, tl:tl + 2, 0:129],
                                         start=(pi == 0), stop=(pi == NPAIR - 1),
                                         perf_mode=DR)
                    den = p3o.tile([128, 1], dt.float32, tag="den", name=f"den3_{j}")
                    nc.vector.tensor_scalar(den[:], accF[:, 128:129], SH2,
                                            SH2 * 1e-6, AOT.mult, AOT.add)
                    rcp = p3o.tile([128, 1], dt.float32, tag="rcp", name=f"rcp3_{j}")
                    nc.vector.reciprocal(rcp[:], den[:])
                    ov = p3o.tile([128, 128], dt.float32, tag="ov", name=f"ov{j}")
                    nc.scalar.mul(ov[:], accF[:, 0:128], rcp[:, 0:1])
                    nc.vector.tensor_tensor(ov[:], ov[:], b2_sb[:], AOT.add)
                    ob = p3o.tile([128, 128], dt.bfloat16, tag="ob", name=f"ob{j}")
                    nc.scalar.activation(ob[:], ov[:], AFT.Relu)
                    nc.tensor.matmul(pool_ps[:], bh[:], ob[:],
                                     start=(j == 0), stop=(j == NBLK - 1))
                pc = p3o.tile([NG, 128], dt.float32, tag="pc")
                nc.vector.tensor_copy(pc[:], pool_ps[:])
                nc.sync.dma_start(pout[:], pc[:])

    nc.compile()
    return nc


def prepare_inputs(inputs, cfg):
    """Host-side sharding/layout. Returns in_maps (one dict per core)."""
    IN_DIM, HOG = cfg["in_dim"], cfg["hog"]
    N, NPC, NBLK, NT1, NG = cfg["n"], cfg["npc"], cfg["nblk"], cfg["nt1"], cfg["ng"]
    NPAD = NPC * NCORES

    x = np.asarray(inputs["x"], np.float32)
    ei = np.asarray(inputs["edge_index"])
    batch = np.asarray(inputs["batch"]).astype(np.int64)
    W1 = np.asarray(inputs["W1"], np.float32)
    a_src1 = np.asarray(inputs["a_src1"], np.float32)
    a_dst1 = np.asarray(inputs["a_dst1"], np.float32)
    W2 = np.asarray(inputs["W2"], np.float32)
    a_src2 = np.asarray(inputs["a_src2"], np.float32)
    a_dst2 = np.asarray(inputs["a_dst2"], np.float32)
    attn_W = np.asarray(inputs["attn_W"], np.float32)
    attn_b = np.asarray(inputs["attn_b"], np.float32)
    b1 = np.asarray(inputs["b1"], np.float32)
    b2 = np.asarray(inputs["b2"], np.float32)

    # augmented weights (scale-folded)
    w1aug = np.zeros((IN_DIM, 264), np.float32)
    w1aug[:, 0:128] = W1[:, 0:128] * SH1
    w1aug[:, 129:257] = W1[:, 128:256] * SH1
    w1aug[:, 258] = W1[:, 0:128] @ a_src1[0] * SA
    w1aug[:, 259] = W1[:, 128:256] @ a_src1[1] * SA
    w1aug[:, 260] = W1[:, 0:128] @ a_dst1[0] * SA
    w1aug[:, 261] = W1[:, 128:256] @ a_dst1[1] * SA
    w1aug[:, 262:264] = attn_W * SG
    w2aug = np.zeros((256, 132), np.float32)
    w2aug[:, 0:128] = W2 * SH2
    w2aug[:, 129] = W2 @ a_src2[0] * SA
    w2aug[:, 130] = W2 @ a_dst2[0] * SA

    xT = np.zeros((IN_DIM, NPAD), FP8)
    xT[:, :N] = np.ascontiguousarray(x.T).astype(FP8)

    # edges sorted by destination, self loops appended
    idt = ei.dtype
    src = np.concatenate([ei[0], np.arange(N, dtype=idt)]).astype(np.int64)
    dst = np.concatenate([ei[1], np.arange(N, dtype=idt)]).astype(np.int64)
    order = np.argsort(dst, kind="stable")
    src_s, dst_s = src[order], dst[order]
    nblk_g = NPAD // 128
    L = NT1 * 128
    cnt = np.bincount(dst_s // 128, minlength=nblk_g)
    assert cnt.max() <= L, (cnt.max(), L)
    offs = np.concatenate([[0], np.cumsum(cnt)])
    sidx_all = np.zeros((nblk_g, L), np.int64)
    dloc_all = np.full((nblk_g, L), -1.0, np.float32)
    for b in range(nblk_g):
        s, e = offs[b], offs[b + 1]
        n = e - s
        sidx_all[b, :n] = src_s[s:e]
        dloc_all[b, :n] = (dst_s[s:e] - 128 * b).astype(np.float32)

    # full one-hot (transposed): otp[b, dl, e] = 1
    otp_all = np.zeros((nblk_g, 128, L), FP8)
    # Sa one-hot in edge-tile layout: saT[b, e%128, (e//128)*128 + dl] = 1
    saT_all = np.zeros((nblk_g, 128, L), FP8)
    eidx = np.arange(L)
    for b in range(nblk_g):
        m = dloc_all[b] >= 0
        dlv = dloc_all[b][m].astype(np.int64)
        ev = eidx[m]
        otp_all[b, dlv, ev] = 1.0
        saT_all[b, ev % 128, (ev // 128) * 128 + dlv] = 1.0

    bh_all = np.zeros((nblk_g, 128, NG), np.float32)
    for b in range(nblk_g):
        base = 128 * b
        hi = min(N - base, 128)
        if hi > 0:
            bh_all[b, np.arange(hi), batch[base:base + hi]] = 1.0

    ident_t = np.eye(128, dtype=np.float32).astype(BF16)
    b1b = np.tile(b1[None, :], (128, 1)).astype(np.float32)
    b2b = np.tile(b2[None, :], (128, 1)).astype(np.float32)
    abb = np.tile(attn_b[None, :], (128, 1)).astype(np.float32)

    # chunk-major AllGather layout with chunk sizes AG_CHUNKS (in tiles):
    # node n (core c, local row r, local tile m=r//128, chunk ci) ->
    #   row 128*(cs*8 + c*cz) + (r - cs*128)
    tile_chunk = np.zeros(NBLK, np.int64)
    chunk_start = np.zeros(len(AG_CHUNKS), np.int64)
    s = 0
    for ci, cz in enumerate(AG_CHUNKS):
        chunk_start[ci] = s
        tile_chunk[s:s + cz] = ci
        s += cz
    n_arr = np.arange(NPAD, dtype=np.int64)
    cc_, rr_ = n_arr // NPC, n_arr % NPC
    mm_ = rr_ // 128
    ci_ = tile_chunk[mm_]
    cs_ = chunk_start[ci_]
    cz_ = np.asarray(AG_CHUNKS, np.int64)[ci_]
    rowmap = 128 * (cs_ * NCORES + cc_ * cz_) + (rr_ - cs_ * 128)
    # h2f chunk-major layout per H2_CHUNKS
    tile_chunk2 = np.zeros(NBLK, np.int64)
    chunk_start2 = np.zeros(len(H2_CHUNKS), np.int64)
    s = 0
    for ci, cz in enumerate(H2_CHUNKS):
        chunk_start2[ci] = s
        tile_chunk2[s:s + cz] = ci
        s += cz
    ci2_ = tile_chunk2[mm_]
    cs2_ = chunk_start2[ci2_]
    cz2_ = np.asarray(H2_CHUNKS, np.int64)[ci2_]
    rowmap2 = 128 * (cs2_ * NCORES + cc_ * cz2_) + (rr_ - cs2_ * 128)
    in_maps = []
    for c in range(NCORES):
        blo, bhi = c * NBLK, (c + 1) * NBLK
        gidx_c = np.stack([
            _wrap_idx(rowmap[sidx_all[b]]) for b in range(blo, bhi)
        ])
        gidx2_c = np.stack([
            _wrap_idx(rowmap2[sidx_all[b]]) for b in range(blo, bhi)
        ])
        in_maps.append({
            "xT": np.ascontiguousarray(xT[:, c * NPC:(c + 1) * NPC]),
            "w1": w1aug.astype(FP8),
            "w2": np.ascontiguousarray(
                w2aug.reshape(2, 128, 132).astype(BF16)),
            "b1b": b1b, "b2b": b2b, "abb": abb,
            "identin": ident_t,
            "gidx": gidx_c,
            "gidx2": gidx2_c,
            "otp": otp_all[blo:bhi],
            "saT": saT_all[blo:bhi],
            "bhot": bh_all[blo:bhi].astype(BF16),
        })
    return in_maps


def _edge_cfg(inputs):
    ei = np.asarray(inputs["edge_index"])
    x = np.asarray(inputs["x"])
    n = x.shape[0]
    npad = -(-n // (128 * NCORES)) * 128 * NCORES
    npc = npad // NCORES
    dst = np.concatenate([ei[1].astype(np.int64), np.arange(n, dtype=np.int64)])
    cnt = np.bincount(dst // 128, minlength=npad // 128)
    nt1 = int(-(-cnt.max() // 128))
    nt1 += nt1 % 2  # even for DoubleRow pairs
    return {
        "n": n, "npc": npc, "nblk": npc // 128, "nt1": nt1,
        "in_dim": x.shape[1], "hog": 4464 if x.shape[1] == 4527 else None,
        "ng": 64,
    }


def kernel(**inputs):
    global LAST_EXEC_NS
    cfg = _edge_cfg(inputs)
    if cfg["hog"] is None:
        raise ValueError("unexpected input width")
    batch = np.asarray(inputs["batch"]).astype(np.int64)
    Wc1 = np.asarray(inputs["Wc1"], np.float32)
    bc1 = np.asarray(inputs["bc1"], np.float32)
    Wc2 = np.asarray(inputs["Wc2"], np.float32)
    bc2 = np.asarray(inputs["bc2"], np.float32)

    in_maps = prepare_inputs(inputs, cfg)
    nc = build_program(cfg)

    profile = os.environ.get("AGAT_PROFILE", "") == "1"
    if profile:
        _install_ntff_hook()
    res = run_bass_kernel_spmd(
        nc, in_maps, core_ids=list(range(NCORES)), trace=profile,
        tmpdir=os.environ.get("AGAT_PROFILE_DIR") or None,
    )
    if profile:
        LAST_EXEC_NS = res.exec_time_ns

    pooled = np.zeros((cfg["ng"], 128), np.float64)
    for c in range(NCORES):
        pooled += res.results[c]["pout"].astype(np.float64)
    cntg = np.bincount(batch, minlength=cfg["ng"]).astype(np.float64)
    pooled = (pooled / np.maximum(cntg, 1.0)[:, None]).astype(np.float32)
    z = np.maximum(pooled @ Wc1 + bc1, 0.0)
    return (z @ Wc2 + bc2).astype(np.float32)
